# revision 4
# baseline (speedup 1.0000x reference)
"""BinConv2d (XNOR-Net style) Trainium2 kernel, 8-core data-parallel.

Layer math (BatchNorm train-mode -> BinActiv -> binary 3x3 conv -> scale by
box-filtered channel-mean magnitudes and per-filter alpha -> relu):

  mu, var: batch stats of x over (N, H, W) per channel      (needs all-reduce)
  xn  = (x - mu) * rsqrt(var + eps) * gamma + beta
  m   = mean_c |xn|;  xb = sign(xn);  Wb = sign(W);  alpha = mean |W| per filter
  y   = conv(xb, Wb, pad=1) + b
  out = relu(y * box3x3(m) * alpha)

Key implementation facts:
  - xb, Wb are exactly representable in bf16; matmuls accumulate fp32 in PSUM,
    so the binary conv result is EXACT integers.
  - sign(xn) = sign(x + t') with t' = beta*sigma/gamma - mu  (gamma > 0), and
    |xn| = s*|x + t'| with s = gamma*rsig folded into the m-matmul weights.
  - 3x3 conv = 9 shifted matmuls over a zero-padded [C, 58*58] flat layout;
    every tap is a pure 1D offset, pad columns absorb edge effects.
"""

import os
import sys

import numpy as np

for _p in ("/opt/trn_rl_repo", "/root/.axon_site/_ro/trn_rl_repo"):
    if os.path.isdir(_p) and _p not in sys.path:
        sys.path.insert(0, _p)

import concourse.bass as bass  # noqa: E402
import concourse.bacc as bacc  # noqa: E402
import concourse.mybir as mybir  # noqa: E402
import concourse.tile as tile  # noqa: E402
from concourse.bass_utils import run_bass_kernel_spmd  # noqa: E402

F32 = mybir.dt.float32
BF16 = mybir.dt.bfloat16
NPBF16 = mybir.dt.np(BF16)
AF = mybir.ActivationFunctionType
ALU = mybir.AluOpType
AX = mybir.AxisListType

EPS = 1e-4
NCORES = 8
P = 128
CIN = 256
COUT = 256
H = 56
W = 56
HP = H + 2          # 58 padded rows
WP = W + 2          # 58 padded cols
IMGP = HP * WP      # 3364 padded pixels / image
NPIX = H * W        # 3136 true pixels / image
MARGIN = 64         # dead zero margin absorbing out-of-image tap reads
CH_ROWS = 8         # output rows per PSUM chunk
NCH = H // CH_ROWS  # 7 chunks
CF = CH_ROWS * W    # 448 compact free elems / chunk
CFP = CH_ROWS * WP  # 464 padded free elems / chunk
KTAPS = 9


def _build(n_local: int):
    """Build the SPMD program for n_local images per core."""
    NL = n_local
    FREEPAD = 2 * MARGIN + NL * IMGP

    nc = bacc.Bacc("TRN2", debug=False, target_bir_lowering=False,
                   num_devices=NCORES)
    x_d = nc.declare_dram_parameter("x", [NL, CIN, H, W], F32, isOutput=False)
    g_d = nc.declare_dram_parameter("gamma", [CIN], F32, isOutput=False)
    bb_d = nc.declare_dram_parameter("beta_bn", [CIN], F32, isOutput=False)
    w_d = nc.declare_dram_parameter("W", [COUT, CIN, 3, 3], F32, isOutput=False)
    b_d = nc.declare_dram_parameter("b", [COUT], F32, isOutput=False)
    id_d = nc.declare_dram_parameter("ident", [P, P], F32, isOutput=False)
    on_d = nc.declare_dram_parameter("ones_bf", [1, P], BF16, isOutput=False)
    tv_d = nc.declare_dram_parameter("tvt", [HP, H], BF16, isOutput=False)
    out_d = nc.declare_dram_parameter("out", [NL, COUT, H, W], F32, isOutput=True)

    with tile.TileContext(nc, num_cores=NCORES) as tc:
        with (
            tc.tile_pool(name="statics", bufs=1) as st,
            tc.tile_pool(name="xw", bufs=3) as xw,
            tc.tile_pool(name="axnp", bufs=4) as axnp,
            tc.tile_pool(name="smalls", bufs=2) as sm,
            tc.tile_pool(name="zp", bufs=3) as zp,
            tc.tile_pool(name="outp", bufs=4) as outp,
            tc.tile_pool(name="ps_conv", bufs=4, space="PSUM") as ps_conv,
            tc.tile_pool(name="ps_small", bufs=2, space="PSUM") as ps_small,
            tc.tile_pool(name="ps_bc", bufs=2, space="PSUM") as ps_bc,
            tc.tile_pool(name="dram", bufs=1, space="DRAM") as dr,
        ):
            # ---------------- static buffers (zeroed pads) ----------------
            xb = []
            for kc in range(2):
                xbt = st.tile([P, FREEPAD], BF16, name=f"xbuf{kc}", tag=f"xbuf{kc}")
                nc.vector.memset(xbt[:], 0.0)
                xb.append(xbt)
            m_flat = []
            for img in range(NL):
                mf = st.tile([1, IMGP], BF16, name=f"mflat{img}", tag=f"mflat{img}")
                nc.vector.memset(mf[:], 0.0)
                m_flat.append(mf)

            # ---------------- host constants ----------------
            ident = st.tile([P, P], F32, name="ident_sb", tag="ident_sb")
            nc.sync.dma_start(ident[:], id_d.ap())
            onesb = st.tile([1, P], BF16, name="onesb_sb", tag="onesb_sb")
            nc.sync.dma_start(onesb[:], on_d.ap())
            tvt = st.tile([HP, H], BF16, name="tvt_sb", tag="tvt_sb")
            nc.sync.dma_start(tvt[:], tv_d.ap())

            gam, bet = [], []
            for kc in range(2):
                g = st.tile([P, 1], F32, name=f"gam{kc}", tag=f"gam{kc}")
                nc.sync.dma_start(g[:], g_d.ap()[kc * P:(kc + 1) * P][:, None])
                gam.append(g)
                be = st.tile([P, 1], F32, name=f"bet{kc}", tag=f"bet{kc}")
                nc.sync.dma_start(be[:], bb_d.ap()[kc * P:(kc + 1) * P][:, None])
                bet.append(be)
            bsb = []
            for oc in range(2):
                bt = st.tile([P, 1], F32, name=f"bsb{oc}", tag=f"bsb{oc}")
                nc.sync.dma_start(bt[:], b_d.ap()[oc * P:(oc + 1) * P][:, None])
                bsb.append(bt)

            # ---------------- weight prep ----------------
            w_nat = []
            for oc in range(2):
                wn = xw.tile([P, NPIX], F32, name="w_nat", tag="xw")
                nc.sync.dma_start(
                    wn[:, 0:CIN * KTAPS],
                    w_d.ap()[oc * P:(oc + 1) * P].rearrange("o c kh kw -> o (c kh kw)"),
                )
                w_nat.append(wn)
            alpha_sc, ab = [], []
            for oc in range(2):
                araw = st.tile([P, 1], F32, name=f"alph_raw{oc}", tag=f"alph_raw{oc}")
                nc.vector.tensor_reduce(
                    araw[:], w_nat[oc][:, 0:CIN * KTAPS], axis=AX.X, op=ALU.add,
                    apply_absolute_value=True,
                )
                asc = st.tile([P, 1], F32, name=f"alph{oc}", tag=f"alph{oc}")
                nc.vector.tensor_scalar_mul(asc[:], araw[:], 1.0 / (CIN * KTAPS))
                alpha_sc.append(asc)
                abt = st.tile([P, 1], F32, name=f"ab{oc}", tag=f"ab{oc}")
                nc.vector.tensor_mul(abt[:], asc[:], bsb[oc][:])
                ab.append(abt)

            wb = []
            for kc in range(2):
                wbt = st.tile([P, KTAPS * COUT], BF16, name=f"wbin{kc}", tag=f"wbin{kc}")
                wb.append(wbt)
            for oc in range(2):
                wv = w_nat[oc][:, 0:CIN * KTAPS].rearrange("o (c t) -> o c t", t=KTAPS)
                for kc in range(2):
                    for tap in range(KTAPS):
                        psT = ps_conv.tile([P, P], F32, name="psT", tag="ps_conv")
                        nc.tensor.transpose(psT[:], wv[:, kc * P:(kc + 1) * P, tap], ident[:])
                        nc.scalar.activation(
                            wb[kc][:, tap * COUT + oc * P: tap * COUT + oc * P + P],
                            psT[:], AF.Sign,
                        )

            # ---------------- pass 1: BN statistics ----------------
            stats = []
            for kc in range(2):
                sb = st.tile([P, NL * NCH * 6], F32, name=f"stats{kc}", tag=f"stats{kc}")
                stats.append(sb)
            for img in range(NL):
                for kc in range(2):
                    xt = xw.tile([P, NPIX], F32, name="xt", tag="xw")
                    nc.sync.dma_start(
                        xt[:], x_d.ap()[img, kc * P:(kc + 1) * P].rearrange("c h w -> c (h w)")
                    )
                    for g in range(NCH):
                        col = (img * NCH + g) * 6
                        nc.vector.bn_stats(
                            stats[kc][:, col:col + 6],
                            xt[:, g * CF:(g + 1) * CF],
                        )
            pay = st.tile([P, 4], F32, name="pay", tag="pay")
            for kc in range(2):
                ag = st.tile([P, 2], F32, name=f"agg{kc}", tag=f"agg{kc}")
                nc.vector.bn_aggr(ag[:], stats[kc][:])
                nc.vector.tensor_copy(pay[:, 2 * kc:2 * kc + 1], ag[:, 0:1])
                msq = st.tile([P, 1], F32, name=f"msq{kc}", tag=f"msq{kc}")
                nc.vector.tensor_mul(msq[:], ag[:, 0:1], ag[:, 0:1])
                # E[x^2] per core = var + mean^2
                nc.vector.tensor_add(pay[:, 2 * kc + 1:2 * kc + 2], ag[:, 1:2], msq[:])

            cc_in = dr.tile([P, 4], F32, name="cc_in", tag="cc_in")
            cc_out = dr.tile([P, 4], F32, name="cc_out", tag="cc_out", addr_space="Shared")
            nc.sync.dma_start(cc_in[:], pay[:])
            nc.gpsimd.collective_compute(
                "AllReduce", ALU.add,
                replica_groups=[list(range(NCORES))],
                ins=[cc_in.opt()], outs=[cc_out.opt()],
            )
            arsb = st.tile([P, 4], F32, name="arsb", tag="arsb")
            nc.sync.dma_start(arsb[:], cc_out[:])

            # ---------------- global BN scalars ----------------
            epsc = st.tile([P, 1], F32, name="epsc", tag="epsc")
            nc.vector.memset(epsc[:], EPS)
            svec_bf, tprime = [], []
            for kc in range(2):
                mu = st.tile([P, 1], F32, name=f"mu{kc}", tag=f"mu{kc}")
                nc.vector.tensor_scalar_mul(mu[:], arsb[:, 2 * kc:2 * kc + 1], 1.0 / NCORES)
                ex2 = st.tile([P, 1], F32, name=f"ex2{kc}", tag=f"ex2{kc}")
                nc.vector.tensor_scalar_mul(ex2[:], arsb[:, 2 * kc + 1:2 * kc + 2], 1.0 / NCORES)
                msq2 = st.tile([P, 1], F32, name=f"musq{kc}", tag=f"musq{kc}")
                nc.vector.tensor_mul(msq2[:], mu[:], mu[:])
                var = st.tile([P, 1], F32, name=f"var{kc}", tag=f"var{kc}")
                nc.vector.tensor_sub(var[:], ex2[:], msq2[:])
                sig = st.tile([P, 1], F32, name=f"sig{kc}", tag=f"sig{kc}")
                nc.scalar.activation(sig[:], var[:], AF.Sqrt, bias=epsc[:])
                rsig = st.tile([P, 1], F32, name=f"rsig{kc}", tag=f"rsig{kc}")
                nc.vector.reciprocal(rsig[:], sig[:])
                s = st.tile([P, 1], F32, name=f"s{kc}", tag=f"s{kc}")
                nc.vector.tensor_mul(s[:], gam[kc][:], rsig[:])
                rg = st.tile([P, 1], F32, name=f"rg{kc}", tag=f"rg{kc}")
                nc.vector.reciprocal(rg[:], gam[kc][:])
                tb = st.tile([P, 1], F32, name=f"tb{kc}", tag=f"tb{kc}")
                nc.vector.tensor_mul(tb[:], bet[kc][:], sig[:])
                tb2 = st.tile([P, 1], F32, name=f"tb2{kc}", tag=f"tb2{kc}")
                nc.vector.tensor_mul(tb2[:], tb[:], rg[:])
                tp = st.tile([P, 1], F32, name=f"tp{kc}", tag=f"tp{kc}")
                nc.vector.tensor_sub(tp[:], tb2[:], mu[:])
                tprime.append(tp)
                smf = st.tile([P, 1], F32, name=f"smf{kc}", tag=f"smf{kc}")
                nc.vector.tensor_scalar_mul(smf[:], s[:], 1.0 / CIN)
                smb = st.tile([P, 1], BF16, name=f"smb{kc}", tag=f"smb{kc}")
                nc.vector.tensor_copy(smb[:], smf[:])
                svec_bf.append(smb)

            # ---------------- pass 2: binarize + conv, per image ----------------
            for img in range(NL):
                axn = []
                for kc in range(2):
                    xt2 = xw.tile([P, NPIX], F32, name="xt2", tag="xw")
                    nc.sync.dma_start(
                        xt2[:], x_d.ap()[img, kc * P:(kc + 1) * P].rearrange("c h w -> c (h w)")
                    )
                    ax = axnp.tile([P, NPIX], BF16, name="ax", tag="ax")
                    nc.scalar.activation(ax[:], xt2[:], AF.Abs, bias=tprime[kc][:])
                    xbv = (xb[kc][:, MARGIN + img * IMGP: MARGIN + (img + 1) * IMGP]
                           .rearrange("p (h w) -> p h w", w=WP))
                    nc.scalar.activation(
                        xbv[:, 1:1 + H, 1:1 + W],
                        xt2.rearrange("p (h w) -> p h w", w=W),
                        AF.Sign, bias=tprime[kc][:],
                    )
                    axn.append(ax)

                # channel-mean magnitudes m (weighted column sums via PE)
                for ch in range(NCH):
                    mp = ps_small.tile([1, CF], F32, name="mps", tag="ps_small")
                    nc.tensor.matmul(mp[:], svec_bf[0][:], axn[0][:, ch * CF:(ch + 1) * CF],
                                     start=True, stop=False)
                    nc.tensor.matmul(mp[:], svec_bf[1][:], axn[1][:, ch * CF:(ch + 1) * CF],
                                     start=False, stop=True)
                    mfv = m_flat[img].rearrange("p (h w) -> p h w", w=WP)
                    nc.scalar.activation(
                        mfv[:, 1 + ch * CH_ROWS: 1 + (ch + 1) * CH_ROWS, 1:1 + W],
                        mp.rearrange("p (h w) -> p h w", w=W),
                        AF.Copy,
                    )

                # beta_map = box3x3(m): horizontal on DVE, vertical via banded matmul
                mhw = sm.tile([HP, WP], BF16, name="mhw", tag="mhw")
                nc.sync.dma_start(mhw[:], m_flat[img][:])
                hs = sm.tile([HP, WP], BF16, name="hs", tag="hs")
                nc.vector.tensor_add(hs[:, 1:1 + W], mhw[:, 0:W], mhw[:, 2:2 + W])
                nc.vector.tensor_add(hs[:, 1:1 + W], hs[:, 1:1 + W], mhw[:, 1:1 + W])
                bps = ps_small.tile([H, W], F32, name="bps", tag="ps_small")
                nc.tensor.matmul(bps[:], tvt[:], hs[:, 1:1 + W], start=True, stop=True)
                bhw = sm.tile([H, W], BF16, name="bhw", tag="bhw")
                nc.vector.tensor_copy(bhw[:], bps[:])
                bflat = sm.tile([1, NPIX], BF16, name="bflat", tag="bflat")
                nc.sync.dma_start(bflat[:], bhw[:])

                # binary conv + epilogue
                for ch in range(NCH):
                    bcp = ps_bc.tile([P, CF], F32, name="bcp", tag="ps_bc")
                    nc.tensor.matmul(bcp[:], onesb[:], bflat[:, ch * CF:(ch + 1) * CF],
                                     start=True, stop=True)
                    base = MARGIN + img * IMGP + (1 + ch * CH_ROWS) * WP
                    for oc in range(2):
                        cv = ps_conv.tile([P, CFP], F32, name="cv", tag="ps_conv")
                        k = 0
                        for tap in range(KTAPS):
                            dh, dw = tap // 3, tap % 3
                            off = (dh - 1) * WP + (dw - 1)
                            for kc in range(2):
                                nc.tensor.matmul(
                                    cv[:],
                                    wb[kc][:, tap * COUT + oc * P: tap * COUT + oc * P + P],
                                    xb[kc][:, base + off: base + off + CFP],
                                    start=(k == 0), stop=(k == 2 * KTAPS - 1),
                                )
                                k += 1
                        z = zp.tile([P, CF], F32, name="z", tag="z")
                        cvv = cv.rearrange("p (h w) -> p h w", w=WP)
                        nc.scalar.activation(
                            z.rearrange("p (h w) -> p h w", w=W),
                            cvv[:, :, 1:1 + W],
                            AF.Relu, bias=ab[oc][:], scale=alpha_sc[oc][:],
                        )
                        ot = outp.tile([P, CF], F32, name="ot", tag="ot")
                        nc.vector.tensor_mul(ot[:], z[:], bcp[:])
                        nc.sync.dma_start(
                            out_d.ap()[img, oc * P:(oc + 1) * P,
                                       ch * CH_ROWS:(ch + 1) * CH_ROWS, :],
                            ot.rearrange("p (h w) -> p h w", w=W),
                        )

    nc.compile()
    return nc


_NC_CACHE: dict = {}


def _get_nc(n_local: int):
    if n_local not in _NC_CACHE:
        _NC_CACHE[n_local] = _build(n_local)
    return _NC_CACHE[n_local]


def _host_consts():
    ident = np.eye(P, dtype=np.float32)
    ones_bf = np.ones((1, P), dtype=NPBF16)
    tvt = np.zeros((HP, H), dtype=np.float32)
    for h in range(H):
        tvt[h:h + 3, h] = 1.0 / 9.0
    return ident, ones_bf, tvt.astype(NPBF16)


def _run(inputs: dict, trace: bool = False):
    x = np.ascontiguousarray(np.asarray(inputs["x"], dtype=np.float32))
    gamma = np.ascontiguousarray(np.asarray(inputs["gamma"], dtype=np.float32))
    beta_bn = np.ascontiguousarray(np.asarray(inputs["beta_bn"], dtype=np.float32))
    Wt = np.ascontiguousarray(np.asarray(inputs["W"], dtype=np.float32))
    b = np.ascontiguousarray(np.asarray(inputs["b"], dtype=np.float32))

    n = x.shape[0]
    assert n % NCORES == 0, f"batch {n} not divisible by {NCORES}"
    nl = n // NCORES
    nc = _get_nc(nl)
    ident, ones_bf, tvt = _host_consts()

    in_maps = []
    for i in range(NCORES):
        in_maps.append({
            "x": np.ascontiguousarray(x[i * nl:(i + 1) * nl]),
            "gamma": gamma, "beta_bn": beta_bn, "W": Wt, "b": b,
            "ident": ident, "ones_bf": ones_bf, "tvt": tvt,
        })
    res = run_bass_kernel_spmd(nc, in_maps, core_ids=list(range(NCORES)),
                               trace=trace)
    out = np.concatenate([res.results[i]["out"] for i in range(NCORES)], axis=0)
    return out, res


def kernel(**inputs) -> np.ndarray:
    out, _ = _run(inputs, trace=False)
    return out


def kernel_timed(**inputs):
    out, res = _run(inputs, trace=True)
    return out, res


# revision 10
# speedup vs baseline: 1.1629x; 1.1629x over previous
"""BinConv2d (XNOR-Net style) Trainium2 kernel, 8-core data-parallel.

Layer math (BatchNorm train-mode -> BinActiv -> binary 3x3 conv -> scale by
box-filtered channel-mean magnitudes and per-filter alpha -> relu):

  mu, var: batch stats of x over (N, H, W) per channel      (needs all-reduce)
  xn  = (x - mu) * rsqrt(var + eps) * gamma + beta
  m   = mean_c |xn|;  xb = sign(xn);  Wb = sign(W);  alpha = mean |W| per filter
  y   = conv(xb, Wb, pad=1) + b
  out = relu(y * box3x3(m) * alpha)

Key implementation facts:
  - xb, Wb are exactly representable in bf16; matmuls accumulate fp32 in PSUM,
    so the binary conv result is EXACT integers.
  - sign(xn) = sign(x + t') with t' = beta*sigma/gamma - mu  (gamma > 0), and
    |xn| = s*|x + t'| with s = gamma*rsig folded into the m-matmul weights.
  - 3x3 conv = 9 shifted matmuls over a zero-padded [C, 58*58] flat layout;
    every tap is a pure 1D offset, pad columns absorb edge effects.
"""

import os
import sys

import numpy as np

for _p in ("/opt/trn_rl_repo", "/root/.axon_site/_ro/trn_rl_repo"):
    if os.path.isdir(_p) and _p not in sys.path:
        sys.path.insert(0, _p)

import concourse.bass as bass  # noqa: E402
import concourse.bacc as bacc  # noqa: E402
import concourse.mybir as mybir  # noqa: E402
import concourse.tile as tile  # noqa: E402
from concourse.bass_utils import run_bass_kernel_spmd  # noqa: E402

F32 = mybir.dt.float32
BF16 = mybir.dt.bfloat16
FP8 = mybir.dt.float8e4
NPBF16 = mybir.dt.np(BF16)
AF = mybir.ActivationFunctionType
ALU = mybir.AluOpType
AX = mybir.AxisListType

EPS = 1e-4
NCORES = 8
P = 128
CIN = 256
COUT = 256
H = 56
W = 56
HP = H + 2          # 58 padded rows
WP = W + 2          # 58 padded cols
IMGP = HP * WP      # 3364 padded pixels / image
NPIX = H * W        # 3136 true pixels / image
MARGIN = 64         # dead zero margin absorbing out-of-image tap reads
CH_ROWS = 8         # output rows per PSUM chunk
NCH = H // CH_ROWS  # 7 chunks
CF = CH_ROWS * W    # 448 compact free elems / chunk
CFP = CH_ROWS * WP  # 464 padded free elems / chunk
KTAPS = 9


def _build(n_local: int):
    """Build the SPMD program for n_local images per core."""
    NL = n_local
    FREEPAD = 2 * MARGIN + NL * IMGP

    nc = bacc.Bacc("TRN2", debug=False, target_bir_lowering=False,
                   num_devices=NCORES)
    x_d = nc.declare_dram_parameter("x", [NL, CIN, H, W], F32, isOutput=False)
    g_d = nc.declare_dram_parameter("gamma", [CIN], F32, isOutput=False)
    bb_d = nc.declare_dram_parameter("beta_bn", [CIN], F32, isOutput=False)
    w_d = nc.declare_dram_parameter("W", [COUT, CIN, 3, 3], F32, isOutput=False)
    b_d = nc.declare_dram_parameter("b", [COUT], F32, isOutput=False)
    id_d = nc.declare_dram_parameter("ident", [P, P], F32, isOutput=False)
    on_d = nc.declare_dram_parameter("ones_bf", [1, P], BF16, isOutput=False)
    tv_d = nc.declare_dram_parameter("tvt", [HP, H], BF16, isOutput=False)
    out_d = nc.declare_dram_parameter("out", [NL, COUT, H, W], F32, isOutput=True)

    with tile.TileContext(nc, num_cores=NCORES) as tc:
        with (
            tc.tile_pool(name="statics", bufs=1) as st,
            tc.tile_pool(name="xw", bufs=3) as xw,
            tc.tile_pool(name="axnp", bufs=4) as axnp,
            tc.tile_pool(name="smalls", bufs=2) as sm,
            tc.tile_pool(name="zp", bufs=3) as zp,
            tc.tile_pool(name="outp", bufs=4) as outp,
            tc.tile_pool(name="ps_conv", bufs=4, space="PSUM") as ps_conv,
            tc.tile_pool(name="ps_small", bufs=2, space="PSUM") as ps_small,
            tc.tile_pool(name="ps_bc", bufs=2, space="PSUM") as ps_bc,
            tc.tile_pool(name="dram", bufs=1, space="DRAM") as dr,
        ):
            # ---------------- static buffers (zeroed pads) ----------------
            # xq holds sign(xn) in fp8 for BOTH channel halves: [P, ko=2, FREEPAD]
            xq = st.tile([P, 2 * FREEPAD], FP8, name="xq", tag="xq")
            # zero only the positions the conv taps read but sign() never
            # writes: the dead margins, the pad rows, and the pad cols.
            for ko in range(2):
                kb = ko * FREEPAD
                nc.vector.memset(xq[:, kb:kb + MARGIN], 0.0)
                nc.vector.memset(xq[:, kb + MARGIN + NL * IMGP: kb + FREEPAD], 0.0)
                for img in range(NL):
                    ib = kb + MARGIN + img * IMGP
                    nc.vector.memset(xq[:, ib:ib + WP], 0.0)                # pad row 0
                    nc.vector.memset(xq[:, ib + (HP - 1) * WP: ib + IMGP], 0.0)  # pad row 57
                    colv = (xq[:, ib + WP: ib + (HP - 1) * WP]
                            .rearrange("p (h w) -> p h w", w=WP))
                    nc.vector.memset(colv[:, :, 0:1], 0.0)                  # pad col 0
                    nc.vector.memset(colv[:, :, WP - 1:WP], 0.0)            # pad col 57
            m_flat = []
            for img in range(NL):
                mf = st.tile([1, IMGP], BF16, name=f"mflat{img}", tag=f"mflat{img}")
                nc.vector.memset(mf[:, 0:WP], 0.0)
                nc.vector.memset(mf[:, (HP - 1) * WP:IMGP], 0.0)
                mfv = mf[:, WP:(HP - 1) * WP].rearrange("p (h w) -> p h w", w=WP)
                nc.vector.memset(mfv[:, :, 0:1], 0.0)
                nc.vector.memset(mfv[:, :, WP - 1:WP], 0.0)
                m_flat.append(mf)

            # ---------------- host constants ----------------
            ident = st.tile([P, P], F32, name="ident_sb", tag="ident_sb")
            nc.sync.dma_start(ident[:], id_d.ap())
            onesb = st.tile([1, P], BF16, name="onesb_sb", tag="onesb_sb")
            nc.sync.dma_start(onesb[:], on_d.ap())
            tvt = st.tile([HP, H], BF16, name="tvt_sb", tag="tvt_sb")
            nc.sync.dma_start(tvt[:], tv_d.ap())

            gam, bet = [], []
            for kc in range(2):
                g = st.tile([P, 1], F32, name=f"gam{kc}", tag=f"gam{kc}")
                nc.sync.dma_start(g[:], g_d.ap()[kc * P:(kc + 1) * P][:, None])
                gam.append(g)
                be = st.tile([P, 1], F32, name=f"bet{kc}", tag=f"bet{kc}")
                nc.sync.dma_start(be[:], bb_d.ap()[kc * P:(kc + 1) * P][:, None])
                bet.append(be)
            bsb = []
            for oc in range(2):
                bt = st.tile([P, 1], F32, name=f"bsb{oc}", tag=f"bsb{oc}")
                nc.sync.dma_start(bt[:], b_d.ap()[oc * P:(oc + 1) * P][:, None])
                bsb.append(bt)

            # ---------------- weight prep ----------------
            w_nat = []
            for oc in range(2):
                wn = xw.tile([P, NPIX], F32, name="w_nat", tag="xw")
                nc.sync.dma_start(
                    wn[:, 0:CIN * KTAPS],
                    w_d.ap()[oc * P:(oc + 1) * P].rearrange("o c kh kw -> o (c kh kw)"),
                )
                w_nat.append(wn)
            alpha_sc, ab = [], []
            for oc in range(2):
                araw = st.tile([P, 1], F32, name=f"alph_raw{oc}", tag=f"alph_raw{oc}")
                nc.vector.tensor_reduce(
                    araw[:], w_nat[oc][:, 0:CIN * KTAPS], axis=AX.X, op=ALU.add,
                    apply_absolute_value=True,
                )
                asc = st.tile([P, 1], F32, name=f"alph{oc}", tag=f"alph{oc}")
                nc.vector.tensor_scalar_mul(asc[:], araw[:], 1.0 / (CIN * KTAPS))
                alpha_sc.append(asc)
                abt = st.tile([P, 1], F32, name=f"ab{oc}", tag=f"ab{oc}")
                nc.vector.tensor_mul(abt[:], asc[:], bsb[oc][:])
                ab.append(abt)

            # wq: sign(W) transposed into DoubleRow lhsT layout
            # [P(ki), tap, oc, ko, m] with ko = channel half (c = ko*128+ki)
            wq = st.tile([P, KTAPS * 2 * 2 * P], FP8, name="wq", tag="wq")
            wqv = wq.rearrange("p (t o k m) -> p t o k m", t=KTAPS, o=2, k=2)
            for oc in range(2):
                wv = w_nat[oc][:, 0:CIN * KTAPS].rearrange("o (c t) -> o c t", t=KTAPS)
                for kc in range(2):
                    for tap in range(KTAPS):
                        psT = ps_conv.tile([P, P], F32, name="psT", tag="ps_conv")
                        nc.tensor.transpose(psT[:], wv[:, kc * P:(kc + 1) * P, tap], ident[:])
                        nc.scalar.activation(wqv[:, tap, oc, kc, :], psT[:], AF.Sign)

            # ---------------- pass 1: BN statistics ----------------
            stats = []
            for kc in range(2):
                sb = st.tile([P, NL * NCH * 6], F32, name=f"stats{kc}", tag=f"stats{kc}")
                stats.append(sb)
            for img in range(NL):
                for kc in range(2):
                    xt = xw.tile([P, NPIX], F32, name="xt", tag="xw")
                    nc.sync.dma_start(
                        xt[:], x_d.ap()[img, kc * P:(kc + 1) * P].rearrange("c h w -> c (h w)")
                    )
                    for g in range(NCH):
                        col = (img * NCH + g) * 6
                        nc.vector.bn_stats(
                            stats[kc][:, col:col + 6],
                            xt[:, g * CF:(g + 1) * CF],
                        )
            pay = st.tile([P, 4], F32, name="pay", tag="pay")
            for kc in range(2):
                ag = st.tile([P, 2], F32, name=f"agg{kc}", tag=f"agg{kc}")
                nc.vector.bn_aggr(ag[:], stats[kc][:])
                nc.vector.tensor_copy(pay[:, 2 * kc:2 * kc + 1], ag[:, 0:1])
                msq = st.tile([P, 1], F32, name=f"msq{kc}", tag=f"msq{kc}")
                nc.vector.tensor_mul(msq[:], ag[:, 0:1], ag[:, 0:1])
                # E[x^2] per core = var + mean^2
                nc.vector.tensor_add(pay[:, 2 * kc + 1:2 * kc + 2], ag[:, 1:2], msq[:])

            cc_in = dr.tile([P, 4], F32, name="cc_in", tag="cc_in")
            cc_out = dr.tile([NCORES, P, 4], F32, name="cc_out", tag="cc_out",
                             addr_space="Shared")
            nc.sync.dma_start(cc_in[:], pay[:])
            nc.gpsimd.collective_compute(
                "AllGather", ALU.bypass,
                replica_groups=[list(range(NCORES))],
                ins=[cc_in.opt()], outs=[cc_out.opt()],
            )
            ag_sb = st.tile([P, NCORES * 4], F32, name="ag_sb", tag="ag_sb")
            nc.sync.dma_start(
                ag_sb[:].rearrange("p (r c) -> p r c", c=4),
                cc_out.rearrange("r p c -> p r c"),
            )
            # local sum over the 8 ranks -> same [P, 4] view the AR produced
            arsb = st.tile([P, 4], F32, name="arsb", tag="arsb")
            nc.vector.tensor_reduce(
                arsb[:],
                ag_sb[:].rearrange("p (r c) -> p c r", c=4),
                axis=AX.X, op=ALU.add,
            )

            # ---------------- global BN scalars ----------------
            epsc = st.tile([P, 1], F32, name="epsc", tag="epsc")
            nc.vector.memset(epsc[:], EPS)
            svec_bf, tprime = [], []
            for kc in range(2):
                mu = st.tile([P, 1], F32, name=f"mu{kc}", tag=f"mu{kc}")
                nc.vector.tensor_scalar_mul(mu[:], arsb[:, 2 * kc:2 * kc + 1], 1.0 / NCORES)
                ex2 = st.tile([P, 1], F32, name=f"ex2{kc}", tag=f"ex2{kc}")
                nc.vector.tensor_scalar_mul(ex2[:], arsb[:, 2 * kc + 1:2 * kc + 2], 1.0 / NCORES)
                msq2 = st.tile([P, 1], F32, name=f"musq{kc}", tag=f"musq{kc}")
                nc.vector.tensor_mul(msq2[:], mu[:], mu[:])
                var = st.tile([P, 1], F32, name=f"var{kc}", tag=f"var{kc}")
                nc.vector.tensor_sub(var[:], ex2[:], msq2[:])
                sig = st.tile([P, 1], F32, name=f"sig{kc}", tag=f"sig{kc}")
                nc.scalar.activation(sig[:], var[:], AF.Sqrt, bias=epsc[:])
                rsig = st.tile([P, 1], F32, name=f"rsig{kc}", tag=f"rsig{kc}")
                nc.vector.reciprocal(rsig[:], sig[:])
                s = st.tile([P, 1], F32, name=f"s{kc}", tag=f"s{kc}")
                nc.vector.tensor_mul(s[:], gam[kc][:], rsig[:])
                rg = st.tile([P, 1], F32, name=f"rg{kc}", tag=f"rg{kc}")
                nc.vector.reciprocal(rg[:], gam[kc][:])
                tb = st.tile([P, 1], F32, name=f"tb{kc}", tag=f"tb{kc}")
                nc.vector.tensor_mul(tb[:], bet[kc][:], sig[:])
                tb2 = st.tile([P, 1], F32, name=f"tb2{kc}", tag=f"tb2{kc}")
                nc.vector.tensor_mul(tb2[:], tb[:], rg[:])
                tp = st.tile([P, 1], F32, name=f"tp{kc}", tag=f"tp{kc}")
                nc.vector.tensor_sub(tp[:], tb2[:], mu[:])
                tprime.append(tp)
                smf = st.tile([P, 1], F32, name=f"smf{kc}", tag=f"smf{kc}")
                nc.vector.tensor_scalar_mul(smf[:], s[:], 1.0 / CIN)
                smb = st.tile([P, 1], BF16, name=f"smb{kc}", tag=f"smb{kc}")
                nc.vector.tensor_copy(smb[:], smf[:])
                svec_bf.append(smb)

            # ---------------- pass 2: binarize + conv, per image ----------------
            for img in range(NL):
                axn = []
                for kc in range(2):
                    xt2 = xw.tile([P, NPIX], F32, name="xt2", tag="xw")
                    nc.sync.dma_start(
                        xt2[:], x_d.ap()[img, kc * P:(kc + 1) * P].rearrange("c h w -> c (h w)")
                    )
                    ax = axnp.tile([P, NPIX], BF16, name="ax", tag="ax")
                    nc.scalar.activation(ax[:], xt2[:], AF.Abs, bias=tprime[kc][:])
                    xqv = (xq[:, kc * FREEPAD + MARGIN + img * IMGP:
                              kc * FREEPAD + MARGIN + (img + 1) * IMGP]
                           .rearrange("p (h w) -> p h w", w=WP))
                    nc.scalar.activation(
                        xqv[:, 1:1 + H, 1:1 + W],
                        xt2.rearrange("p (h w) -> p h w", w=W),
                        AF.Sign, bias=tprime[kc][:],
                    )
                    axn.append(ax)

                # channel-mean magnitudes m (weighted column sums via PE)
                for ch in range(NCH):
                    mp = ps_small.tile([1, CF], F32, name="mps", tag="ps_small")
                    nc.tensor.matmul(mp[:], svec_bf[0][:], axn[0][:, ch * CF:(ch + 1) * CF],
                                     start=True, stop=False)
                    nc.tensor.matmul(mp[:], svec_bf[1][:], axn[1][:, ch * CF:(ch + 1) * CF],
                                     start=False, stop=True)
                    mfv = m_flat[img].rearrange("p (h w) -> p h w", w=WP)
                    nc.scalar.activation(
                        mfv[:, 1 + ch * CH_ROWS: 1 + (ch + 1) * CH_ROWS, 1:1 + W],
                        mp.rearrange("p (h w) -> p h w", w=W),
                        AF.Copy,
                    )

                # beta_map = box3x3(m): horizontal on DVE, vertical via banded matmul
                mhw = sm.tile([HP, WP], BF16, name="mhw", tag="mhw")
                nc.sync.dma_start(mhw[:], m_flat[img][:])
                hs = sm.tile([HP, WP], BF16, name="hs", tag="hs")
                nc.vector.tensor_add(hs[:, 1:1 + W], mhw[:, 0:W], mhw[:, 2:2 + W])
                nc.vector.tensor_add(hs[:, 1:1 + W], hs[:, 1:1 + W], mhw[:, 1:1 + W])
                bps = ps_small.tile([H, W], F32, name="bps", tag="ps_small")
                nc.tensor.matmul(bps[:], tvt[:], hs[:, 1:1 + W], start=True, stop=True)
                bhw = sm.tile([H, W], BF16, name="bhw", tag="bhw")
                nc.vector.tensor_copy(bhw[:], bps[:])
                bflat = sm.tile([1, NPIX], BF16, name="bflat", tag="bflat")
                nc.sync.dma_start(bflat[:], bhw[:])

                # binary conv + epilogue
                for ch in range(NCH):
                    bcp = ps_bc.tile([P, CF], F32, name="bcp", tag="ps_bc")
                    nc.tensor.matmul(bcp[:], onesb[:], bflat[:, ch * CF:(ch + 1) * CF],
                                     start=True, stop=True)
                    base = MARGIN + img * IMGP + (1 + ch * CH_ROWS) * WP
                    xq2 = xq[:].rearrange("p (k f) -> p k f", k=2)
                    for oc in range(2):
                        cv = ps_conv.tile([P, CFP], F32, name="cv", tag="ps_conv")
                        for tap in range(KTAPS):
                            dh, dw = tap // 3, tap % 3
                            off = (dh - 1) * WP + (dw - 1)
                            nc.tensor.matmul(
                                cv[:],
                                wqv[:, tap, oc],
                                xq2[:, :, base + off: base + off + CFP],
                                start=(tap == 0), stop=(tap == KTAPS - 1),
                                perf_mode=mybir.MatmulPerfMode.DoubleRow,
                            )
                        z = zp.tile([P, CF], F32, name="z", tag="z")
                        cvv = cv.rearrange("p (h w) -> p h w", w=WP)
                        nc.scalar.activation(
                            z.rearrange("p (h w) -> p h w", w=W),
                            cvv[:, :, 1:1 + W],
                            AF.Relu, bias=ab[oc][:], scale=alpha_sc[oc][:],
                        )
                        ot = outp.tile([P, CF], F32, name="ot", tag="ot")
                        nc.vector.tensor_mul(ot[:], z[:], bcp[:])
                        nc.sync.dma_start(
                            out_d.ap()[img, oc * P:(oc + 1) * P,
                                       ch * CH_ROWS:(ch + 1) * CH_ROWS, :],
                            ot.rearrange("p (h w) -> p h w", w=W),
                        )

    nc.compile()
    return nc


_NC_CACHE: dict = {}


def _get_nc(n_local: int):
    if n_local not in _NC_CACHE:
        _NC_CACHE[n_local] = _build(n_local)
    return _NC_CACHE[n_local]


def _host_consts():
    ident = np.eye(P, dtype=np.float32)
    ones_bf = np.ones((1, P), dtype=NPBF16)
    tvt = np.zeros((HP, H), dtype=np.float32)
    for h in range(H):
        tvt[h:h + 3, h] = 1.0 / 9.0
    return ident, ones_bf, tvt.astype(NPBF16)


def _run(inputs: dict, trace: bool = False):
    x = np.ascontiguousarray(np.asarray(inputs["x"], dtype=np.float32))
    gamma = np.ascontiguousarray(np.asarray(inputs["gamma"], dtype=np.float32))
    beta_bn = np.ascontiguousarray(np.asarray(inputs["beta_bn"], dtype=np.float32))
    Wt = np.ascontiguousarray(np.asarray(inputs["W"], dtype=np.float32))
    b = np.ascontiguousarray(np.asarray(inputs["b"], dtype=np.float32))

    n = x.shape[0]
    assert n % NCORES == 0, f"batch {n} not divisible by {NCORES}"
    nl = n // NCORES
    nc = _get_nc(nl)
    ident, ones_bf, tvt = _host_consts()

    in_maps = []
    for i in range(NCORES):
        in_maps.append({
            "x": np.ascontiguousarray(x[i * nl:(i + 1) * nl]),
            "gamma": gamma, "beta_bn": beta_bn, "W": Wt, "b": b,
            "ident": ident, "ones_bf": ones_bf, "tvt": tvt,
        })
    res = run_bass_kernel_spmd(nc, in_maps, core_ids=list(range(NCORES)),
                               trace=trace)
    out = np.concatenate([res.results[i]["out"] for i in range(NCORES)], axis=0)
    return out, res


def kernel(**inputs) -> np.ndarray:
    out, _ = _run(inputs, trace=False)
    return out


def kernel_timed(**inputs):
    out, res = _run(inputs, trace=True)
    return out, res


# revision 14
# speedup vs baseline: 1.6140x; 1.3879x over previous
"""BinConv2d (XNOR-Net style) Trainium2 kernel, 8-core data-parallel.

Layer math (BatchNorm train-mode -> BinActiv -> binary 3x3 conv -> scale by
box-filtered channel-mean magnitudes and per-filter alpha -> relu):

  mu, var: batch stats of x over (N, H, W) per channel      (needs all-reduce)
  xn  = (x - mu) * rsqrt(var + eps) * gamma + beta
  m   = mean_c |xn|;  xb = sign(xn);  Wb = sign(W);  alpha = mean |W| per filter
  y   = conv(xb, Wb, pad=1) + b
  out = relu(y * box3x3(m) * alpha)

Key implementation facts:
  - xb, Wb are exactly representable in bf16; matmuls accumulate fp32 in PSUM,
    so the binary conv result is EXACT integers.
  - sign(xn) = sign(x + t') with t' = beta*sigma/gamma - mu  (gamma > 0), and
    |xn| = s*|x + t'| with s = gamma*rsig folded into the m-matmul weights.
  - 3x3 conv = 9 shifted matmuls over a zero-padded [C, 58*58] flat layout;
    every tap is a pure 1D offset, pad columns absorb edge effects.
"""

import os
import sys

import numpy as np

for _p in ("/opt/trn_rl_repo", "/root/.axon_site/_ro/trn_rl_repo"):
    if os.path.isdir(_p) and _p not in sys.path:
        sys.path.insert(0, _p)

import concourse.bass as bass  # noqa: E402
import concourse.bacc as bacc  # noqa: E402
import concourse.mybir as mybir  # noqa: E402
import concourse.tile as tile  # noqa: E402
from concourse.bass_utils import run_bass_kernel_spmd  # noqa: E402

F32 = mybir.dt.float32
BF16 = mybir.dt.bfloat16
FP8 = mybir.dt.float8e4
NPBF16 = mybir.dt.np(BF16)
AF = mybir.ActivationFunctionType
ALU = mybir.AluOpType
AX = mybir.AxisListType

EPS = 1e-4
NCORES = 8
P = 128
CIN = 256
COUT = 256
H = 56
W = 56
HP = H + 2          # 58 padded rows
WP = W + 2          # 58 padded cols
IMGP = HP * WP      # 3364 padded pixels / image
NPIX = H * W        # 3136 true pixels / image
MARGIN = 64         # dead zero margin absorbing out-of-image tap reads
CH_ROWS = 8         # output rows per PSUM chunk
NCH = H // CH_ROWS  # 7 chunks
CF = CH_ROWS * W    # 448 compact free elems / chunk
CFP = CH_ROWS * WP  # 464 padded free elems / chunk
KTAPS = 9


def _build(n_local: int):
    """Build the SPMD program for n_local images per core."""
    NL = n_local
    FREEPAD = 2 * MARGIN + NL * IMGP

    nc = bacc.Bacc("TRN2", debug=False, target_bir_lowering=False,
                   num_devices=NCORES)
    x_d = nc.declare_dram_parameter("x", [NL, CIN, H, W], F32, isOutput=False)
    g_d = nc.declare_dram_parameter("gamma", [CIN], F32, isOutput=False)
    bb_d = nc.declare_dram_parameter("beta_bn", [CIN], F32, isOutput=False)
    w_d = nc.declare_dram_parameter("W", [COUT, CIN, 3, 3], F32, isOutput=False)
    b_d = nc.declare_dram_parameter("b", [COUT], F32, isOutput=False)
    id_d = nc.declare_dram_parameter("ident", [P, P], F32, isOutput=False)
    on_d = nc.declare_dram_parameter("ones_bf", [1, P], BF16, isOutput=False)
    tv_d = nc.declare_dram_parameter("tvt", [HP, H], BF16, isOutput=False)
    out_d = nc.declare_dram_parameter("out", [NL, COUT, H, W], F32, isOutput=True)

    with tile.TileContext(nc, num_cores=NCORES) as tc:
        with (
            tc.tile_pool(name="statics", bufs=1) as st,
            tc.tile_pool(name="xw", bufs=5) as xw,
            tc.tile_pool(name="axnp", bufs=4) as axnp,
            tc.tile_pool(name="smalls", bufs=2) as sm,
            tc.tile_pool(name="zp", bufs=3) as zp,
            tc.tile_pool(name="outp", bufs=4) as outp,
            tc.tile_pool(name="ps_conv", bufs=4, space="PSUM") as ps_conv,
            tc.tile_pool(name="ps_small", bufs=2, space="PSUM") as ps_small,
            tc.tile_pool(name="ps_bc", bufs=2, space="PSUM") as ps_bc,
            tc.tile_pool(name="dram", bufs=1, space="DRAM") as dr,
        ):
            # ---------------- static buffers (zeroed pads) ----------------
            # xq holds sign(xn) in fp8 for BOTH channel halves: [P, ko=2, FREEPAD]
            xq = st.tile([P, 2 * FREEPAD], FP8, name="xq", tag="xq")
            # zero only the positions the conv taps read but sign() never
            # writes: the dead margins, the pad rows, and the pad cols.
            for ko in range(2):
                kb = ko * FREEPAD
                nc.vector.memset(xq[:, kb:kb + MARGIN], 0.0)
                nc.vector.memset(xq[:, kb + MARGIN + NL * IMGP: kb + FREEPAD], 0.0)
                for img in range(NL):
                    ib = kb + MARGIN + img * IMGP
                    nc.vector.memset(xq[:, ib:ib + WP], 0.0)                # pad row 0
                    nc.vector.memset(xq[:, ib + (HP - 1) * WP: ib + IMGP], 0.0)  # pad row 57
                    colv = (xq[:, ib + WP: ib + (HP - 1) * WP]
                            .rearrange("p (h w) -> p h w", w=WP))
                    nc.vector.memset(colv[:, :, 0:1], 0.0)                  # pad col 0
                    nc.vector.memset(colv[:, :, WP - 1:WP], 0.0)            # pad col 57
            m_flat = []
            for img in range(NL):
                mf = st.tile([1, IMGP], BF16, name=f"mflat{img}", tag=f"mflat{img}")
                nc.vector.memset(mf[:, 0:WP], 0.0)
                nc.vector.memset(mf[:, (HP - 1) * WP:IMGP], 0.0)
                mfv = mf[:, WP:(HP - 1) * WP].rearrange("p (h w) -> p h w", w=WP)
                nc.vector.memset(mfv[:, :, 0:1], 0.0)
                nc.vector.memset(mfv[:, :, WP - 1:WP], 0.0)
                m_flat.append(mf)

            # ---------------- warmup collective ----------------
            # The first collective pays ncfw setup + absorbs cross-core start
            # skew; run a throwaway AllGather immediately so the real one
            # (after BN stats) finds warm ncfw state and synchronized peers.
            wu_sb = st.tile([1, 8], F32, name="wu_sb", tag="wu_sb")
            nc.vector.memset(wu_sb[:], 0.0)
            wu_in = dr.tile([1, 8], F32, name="wu_in", tag="wu_in")
            wu_out = dr.tile([NCORES, 1, 8], F32, name="wu_out", tag="wu_out",
                             addr_space="Shared")
            nc.sync.dma_start(wu_in[:], wu_sb[:])
            nc.gpsimd.collective_compute(
                "AllGather", ALU.bypass,
                replica_groups=[list(range(NCORES))],
                ins=[wu_in.opt()], outs=[wu_out.opt()],
            )

            # ---------------- host constants ----------------
            ident = st.tile([P, P], F32, name="ident_sb", tag="ident_sb")
            nc.sync.dma_start(ident[:], id_d.ap())
            onesb = st.tile([1, P], BF16, name="onesb_sb", tag="onesb_sb")
            nc.sync.dma_start(onesb[:], on_d.ap())
            tvt = st.tile([HP, H], BF16, name="tvt_sb", tag="tvt_sb")
            nc.sync.dma_start(tvt[:], tv_d.ap())

            gam, bet = [], []
            for kc in range(2):
                g = st.tile([P, 1], F32, name=f"gam{kc}", tag=f"gam{kc}")
                nc.sync.dma_start(g[:], g_d.ap()[kc * P:(kc + 1) * P][:, None])
                gam.append(g)
                be = st.tile([P, 1], F32, name=f"bet{kc}", tag=f"bet{kc}")
                nc.sync.dma_start(be[:], bb_d.ap()[kc * P:(kc + 1) * P][:, None])
                bet.append(be)
            bsb = []
            for oc in range(2):
                bt = st.tile([P, 1], F32, name=f"bsb{oc}", tag=f"bsb{oc}")
                nc.sync.dma_start(bt[:], b_d.ap()[oc * P:(oc + 1) * P][:, None])
                bsb.append(bt)

            # ---------------- weight prep ----------------
            w_nat = []
            for oc in range(2):
                wn = xw.tile([P, NPIX], F32, name="w_nat", tag="xw")
                nc.sync.dma_start(
                    wn[:, 0:CIN * KTAPS],
                    w_d.ap()[oc * P:(oc + 1) * P].rearrange("o c kh kw -> o (c kh kw)"),
                )
                w_nat.append(wn)
            alpha_sc, ab = [], []
            for oc in range(2):
                araw = st.tile([P, 1], F32, name=f"alph_raw{oc}", tag=f"alph_raw{oc}")
                nc.vector.tensor_reduce(
                    araw[:], w_nat[oc][:, 0:CIN * KTAPS], axis=AX.X, op=ALU.add,
                    apply_absolute_value=True,
                )
                asc = st.tile([P, 1], F32, name=f"alph{oc}", tag=f"alph{oc}")
                nc.vector.tensor_scalar_mul(asc[:], araw[:], 1.0 / (CIN * KTAPS))
                alpha_sc.append(asc)
                abt = st.tile([P, 1], F32, name=f"ab{oc}", tag=f"ab{oc}")
                nc.vector.tensor_mul(abt[:], asc[:], bsb[oc][:])
                ab.append(abt)

            # wq: sign(W) transposed into DoubleRow lhsT layout
            # [P(ki), tap, oc, ko, m] with ko = channel half (c = ko*128+ki)
            wq = st.tile([P, KTAPS * 2 * 2 * P], FP8, name="wq", tag="wq")
            wqv = wq.rearrange("p (t o k m) -> p t o k m", t=KTAPS, o=2, k=2)
            for oc in range(2):
                wv = w_nat[oc][:, 0:CIN * KTAPS].rearrange("o (c t) -> o c t", t=KTAPS)
                for kc in range(2):
                    for tap in range(KTAPS):
                        psT = ps_conv.tile([P, P], F32, name="psT", tag="ps_conv")
                        nc.tensor.transpose(psT[:], wv[:, kc * P:(kc + 1) * P, tap], ident[:])
                        nc.scalar.activation(wqv[:, tap, oc, kc, :], psT[:], AF.Sign)

            # ---------------- pass 1: BN statistics ----------------
            stats = []
            for kc in range(2):
                sb = st.tile([P, NL * NCH * 6], F32, name=f"stats{kc}", tag=f"stats{kc}")
                stats.append(sb)
            for img in range(NL):
                for kc in range(2):
                    xt = xw.tile([P, NPIX], F32, name="xt", tag="xw")
                    nc.sync.dma_start(
                        xt[:], x_d.ap()[img, kc * P:(kc + 1) * P].rearrange("c h w -> c (h w)")
                    )
                    for g in range(NCH):
                        col = (img * NCH + g) * 6
                        nc.vector.bn_stats(
                            stats[kc][:, col:col + 6],
                            xt[:, g * CF:(g + 1) * CF],
                        )
            pay = st.tile([P, 4], F32, name="pay", tag="pay")
            for kc in range(2):
                ag = st.tile([P, 2], F32, name=f"agg{kc}", tag=f"agg{kc}")
                nc.vector.bn_aggr(ag[:], stats[kc][:])
                nc.vector.tensor_copy(pay[:, 2 * kc:2 * kc + 1], ag[:, 0:1])
                msq = st.tile([P, 1], F32, name=f"msq{kc}", tag=f"msq{kc}")
                nc.vector.tensor_mul(msq[:], ag[:, 0:1], ag[:, 0:1])
                # E[x^2] per core = var + mean^2
                nc.vector.tensor_add(pay[:, 2 * kc + 1:2 * kc + 2], ag[:, 1:2], msq[:])

            cc_in = dr.tile([P, 4], F32, name="cc_in", tag="cc_in")
            cc_out = dr.tile([NCORES, P, 4], F32, name="cc_out", tag="cc_out",
                             addr_space="Shared")
            nc.sync.dma_start(cc_in[:], pay[:])
            nc.gpsimd.collective_compute(
                "AllGather", ALU.bypass,
                replica_groups=[list(range(NCORES))],
                ins=[cc_in.opt()], outs=[cc_out.opt()],
            )
            ag_sb = st.tile([P, NCORES * 4], F32, name="ag_sb", tag="ag_sb")
            nc.sync.dma_start(
                ag_sb[:].rearrange("p (r c) -> p r c", c=4),
                cc_out.rearrange("r p c -> p r c"),
            )
            # local sum over the 8 ranks -> same [P, 4] view the AR produced
            arsb = st.tile([P, 4], F32, name="arsb", tag="arsb")
            nc.vector.tensor_reduce(
                arsb[:],
                ag_sb[:].rearrange("p (r c) -> p c r", c=4),
                axis=AX.X, op=ALU.add,
            )

            # ---------------- global BN scalars ----------------
            epsc = st.tile([P, 1], F32, name="epsc", tag="epsc")
            nc.vector.memset(epsc[:], EPS)
            svec_bf, tprime = [], []
            for kc in range(2):
                mu = st.tile([P, 1], F32, name=f"mu{kc}", tag=f"mu{kc}")
                nc.vector.tensor_scalar_mul(mu[:], arsb[:, 2 * kc:2 * kc + 1], 1.0 / NCORES)
                ex2 = st.tile([P, 1], F32, name=f"ex2{kc}", tag=f"ex2{kc}")
                nc.vector.tensor_scalar_mul(ex2[:], arsb[:, 2 * kc + 1:2 * kc + 2], 1.0 / NCORES)
                msq2 = st.tile([P, 1], F32, name=f"musq{kc}", tag=f"musq{kc}")
                nc.vector.tensor_mul(msq2[:], mu[:], mu[:])
                var = st.tile([P, 1], F32, name=f"var{kc}", tag=f"var{kc}")
                nc.vector.tensor_sub(var[:], ex2[:], msq2[:])
                sig = st.tile([P, 1], F32, name=f"sig{kc}", tag=f"sig{kc}")
                nc.scalar.activation(sig[:], var[:], AF.Sqrt, bias=epsc[:])
                rsig = st.tile([P, 1], F32, name=f"rsig{kc}", tag=f"rsig{kc}")
                nc.vector.reciprocal(rsig[:], sig[:])
                s = st.tile([P, 1], F32, name=f"s{kc}", tag=f"s{kc}")
                nc.vector.tensor_mul(s[:], gam[kc][:], rsig[:])
                rg = st.tile([P, 1], F32, name=f"rg{kc}", tag=f"rg{kc}")
                nc.vector.reciprocal(rg[:], gam[kc][:])
                tb = st.tile([P, 1], F32, name=f"tb{kc}", tag=f"tb{kc}")
                nc.vector.tensor_mul(tb[:], bet[kc][:], sig[:])
                tb2 = st.tile([P, 1], F32, name=f"tb2{kc}", tag=f"tb2{kc}")
                nc.vector.tensor_mul(tb2[:], tb[:], rg[:])
                tp = st.tile([P, 1], F32, name=f"tp{kc}", tag=f"tp{kc}")
                nc.vector.tensor_sub(tp[:], tb2[:], mu[:])
                tprime.append(tp)
                smf = st.tile([P, 1], F32, name=f"smf{kc}", tag=f"smf{kc}")
                nc.vector.tensor_scalar_mul(smf[:], s[:], 1.0 / CIN)
                smb = st.tile([P, 1], BF16, name=f"smb{kc}", tag=f"smb{kc}")
                nc.vector.tensor_copy(smb[:], smf[:])
                svec_bf.append(smb)

            # ---------------- pass 2: binarize + conv, per image ----------------
            for img in range(NL):
                axn = []
                for kc in range(2):
                    xt2 = xw.tile([P, NPIX], F32, name="xt2", tag="xw")
                    nc.sync.dma_start(
                        xt2[:], x_d.ap()[img, kc * P:(kc + 1) * P].rearrange("c h w -> c (h w)")
                    )
                    ax = axnp.tile([P, NPIX], BF16, name="ax", tag="ax")
                    nc.scalar.activation(ax[:], xt2[:], AF.Abs, bias=tprime[kc][:])
                    xqv = (xq[:, kc * FREEPAD + MARGIN + img * IMGP:
                              kc * FREEPAD + MARGIN + (img + 1) * IMGP]
                           .rearrange("p (h w) -> p h w", w=WP))
                    nc.scalar.activation(
                        xqv[:, 1:1 + H, 1:1 + W],
                        xt2.rearrange("p (h w) -> p h w", w=W),
                        AF.Sign, bias=tprime[kc][:],
                    )
                    axn.append(ax)

                # channel-mean magnitudes m (weighted column sums via PE)
                for ch in range(NCH):
                    mp = ps_small.tile([1, CF], F32, name="mps", tag="ps_small")
                    nc.tensor.matmul(mp[:], svec_bf[0][:], axn[0][:, ch * CF:(ch + 1) * CF],
                                     start=True, stop=False)
                    nc.tensor.matmul(mp[:], svec_bf[1][:], axn[1][:, ch * CF:(ch + 1) * CF],
                                     start=False, stop=True)
                    mfv = m_flat[img].rearrange("p (h w) -> p h w", w=WP)
                    nc.scalar.activation(
                        mfv[:, 1 + ch * CH_ROWS: 1 + (ch + 1) * CH_ROWS, 1:1 + W],
                        mp.rearrange("p (h w) -> p h w", w=W),
                        AF.Copy,
                    )

                # beta_map = box3x3(m): horizontal on DVE, vertical via banded matmul
                mhw = sm.tile([HP, WP], BF16, name="mhw", tag="mhw")
                nc.sync.dma_start(mhw[:], m_flat[img][:])
                hs = sm.tile([HP, WP], BF16, name="hs", tag="hs")
                nc.vector.tensor_add(hs[:, 1:1 + W], mhw[:, 0:W], mhw[:, 2:2 + W])
                nc.vector.tensor_add(hs[:, 1:1 + W], hs[:, 1:1 + W], mhw[:, 1:1 + W])
                bps = ps_small.tile([H, W], F32, name="bps", tag="ps_small")
                nc.tensor.matmul(bps[:], tvt[:], hs[:, 1:1 + W], start=True, stop=True)
                bhw = sm.tile([H, W], BF16, name="bhw", tag="bhw")
                nc.vector.tensor_copy(bhw[:], bps[:])
                bflat = sm.tile([1, NPIX], BF16, name="bflat", tag="bflat")
                nc.sync.dma_start(bflat[:], bhw[:])

                # binary conv + epilogue
                for ch in range(NCH):
                    bcp = ps_bc.tile([P, CF], F32, name="bcp", tag="ps_bc")
                    nc.tensor.matmul(bcp[:], onesb[:], bflat[:, ch * CF:(ch + 1) * CF],
                                     start=True, stop=True)
                    base = MARGIN + img * IMGP + (1 + ch * CH_ROWS) * WP
                    xq2 = xq[:].rearrange("p (k f) -> p k f", k=2)
                    for oc in range(2):
                        cv = ps_conv.tile([P, CFP], F32, name="cv", tag="ps_conv")
                        for tap in range(KTAPS):
                            dh, dw = tap // 3, tap % 3
                            off = (dh - 1) * WP + (dw - 1)
                            nc.tensor.matmul(
                                cv[:],
                                wqv[:, tap, oc],
                                xq2[:, :, base + off: base + off + CFP],
                                start=(tap == 0), stop=(tap == KTAPS - 1),
                                perf_mode=mybir.MatmulPerfMode.DoubleRow,
                            )
                        z = zp.tile([P, CF], F32, name="z", tag="z")
                        cvv = cv.rearrange("p (h w) -> p h w", w=WP)
                        nc.scalar.activation(
                            z.rearrange("p (h w) -> p h w", w=W),
                            cvv[:, :, 1:1 + W],
                            AF.Relu, bias=ab[oc][:], scale=alpha_sc[oc][:],
                        )
                        ot = outp.tile([P, CF], F32, name="ot", tag="ot")
                        nc.vector.tensor_mul(ot[:], z[:], bcp[:])
                        nc.sync.dma_start(
                            out_d.ap()[img, oc * P:(oc + 1) * P,
                                       ch * CH_ROWS:(ch + 1) * CH_ROWS, :],
                            ot.rearrange("p (h w) -> p h w", w=W),
                        )

    nc.compile()
    return nc


_NC_CACHE: dict = {}


def _get_nc(n_local: int):
    if n_local not in _NC_CACHE:
        _NC_CACHE[n_local] = _build(n_local)
    return _NC_CACHE[n_local]


def _host_consts():
    ident = np.eye(P, dtype=np.float32)
    ones_bf = np.ones((1, P), dtype=NPBF16)
    tvt = np.zeros((HP, H), dtype=np.float32)
    for h in range(H):
        tvt[h:h + 3, h] = 1.0 / 9.0
    return ident, ones_bf, tvt.astype(NPBF16)


def _run(inputs: dict, trace: bool = False):
    x = np.ascontiguousarray(np.asarray(inputs["x"], dtype=np.float32))
    gamma = np.ascontiguousarray(np.asarray(inputs["gamma"], dtype=np.float32))
    beta_bn = np.ascontiguousarray(np.asarray(inputs["beta_bn"], dtype=np.float32))
    Wt = np.ascontiguousarray(np.asarray(inputs["W"], dtype=np.float32))
    b = np.ascontiguousarray(np.asarray(inputs["b"], dtype=np.float32))

    n = x.shape[0]
    assert n % NCORES == 0, f"batch {n} not divisible by {NCORES}"
    nl = n // NCORES
    nc = _get_nc(nl)
    ident, ones_bf, tvt = _host_consts()

    in_maps = []
    for i in range(NCORES):
        in_maps.append({
            "x": np.ascontiguousarray(x[i * nl:(i + 1) * nl]),
            "gamma": gamma, "beta_bn": beta_bn, "W": Wt, "b": b,
            "ident": ident, "ones_bf": ones_bf, "tvt": tvt,
        })
    res = run_bass_kernel_spmd(nc, in_maps, core_ids=list(range(NCORES)),
                               trace=trace)
    out = np.concatenate([res.results[i]["out"] for i in range(NCORES)], axis=0)
    return out, res


def kernel(**inputs) -> np.ndarray:
    out, _ = _run(inputs, trace=False)
    return out


def kernel_timed(**inputs):
    out, res = _run(inputs, trace=True)
    return out, res


# revision 15
# speedup vs baseline: 1.6161x; 1.0013x over previous
"""BinConv2d (XNOR-Net style) Trainium2 kernel, 8-core data-parallel.

Layer math (BatchNorm train-mode -> BinActiv -> binary 3x3 conv -> scale by
box-filtered channel-mean magnitudes and per-filter alpha -> relu):

  mu, var: batch stats of x over (N, H, W) per channel      (needs all-reduce)
  xn  = (x - mu) * rsqrt(var + eps) * gamma + beta
  m   = mean_c |xn|;  xb = sign(xn);  Wb = sign(W);  alpha = mean |W| per filter
  y   = conv(xb, Wb, pad=1) + b
  out = relu(y * box3x3(m) * alpha)

Key implementation facts:
  - xb, Wb are exactly representable in bf16; matmuls accumulate fp32 in PSUM,
    so the binary conv result is EXACT integers.
  - sign(xn) = sign(x + t') with t' = beta*sigma/gamma - mu  (gamma > 0), and
    |xn| = s*|x + t'| with s = gamma*rsig folded into the m-matmul weights.
  - 3x3 conv = 9 shifted matmuls over a zero-padded [C, 58*58] flat layout;
    every tap is a pure 1D offset, pad columns absorb edge effects.
"""

import os
import sys

import numpy as np

for _p in ("/opt/trn_rl_repo", "/root/.axon_site/_ro/trn_rl_repo"):
    if os.path.isdir(_p) and _p not in sys.path:
        sys.path.insert(0, _p)

import concourse.bass as bass  # noqa: E402
import concourse.bacc as bacc  # noqa: E402
import concourse.mybir as mybir  # noqa: E402
import concourse.tile as tile  # noqa: E402
from concourse.bass_utils import run_bass_kernel_spmd  # noqa: E402

F32 = mybir.dt.float32
BF16 = mybir.dt.bfloat16
FP8 = mybir.dt.float8e4
NPBF16 = mybir.dt.np(BF16)
AF = mybir.ActivationFunctionType
ALU = mybir.AluOpType
AX = mybir.AxisListType

EPS = 1e-4
NCORES = 8
P = 128
CIN = 256
COUT = 256
H = 56
W = 56
HP = H + 2          # 58 padded rows
WP = W + 2          # 58 padded cols
IMGP = HP * WP      # 3364 padded pixels / image
NPIX = H * W        # 3136 true pixels / image
MARGIN = 64         # dead zero margin absorbing out-of-image tap reads
CH_ROWS = 8         # output rows per PSUM chunk
NCH = H // CH_ROWS  # 7 chunks
CF = CH_ROWS * W    # 448 compact free elems / chunk
CFP = CH_ROWS * WP  # 464 padded free elems / chunk
KTAPS = 9


def _build(n_local: int):
    """Build the SPMD program for n_local images per core."""
    NL = n_local
    FREEPAD = 2 * MARGIN + NL * IMGP

    nc = bacc.Bacc("TRN2", debug=False, target_bir_lowering=False,
                   num_devices=NCORES)
    x_d = nc.declare_dram_parameter("x", [NL, CIN, H, W], F32, isOutput=False)
    g_d = nc.declare_dram_parameter("gamma", [CIN], F32, isOutput=False)
    bb_d = nc.declare_dram_parameter("beta_bn", [CIN], F32, isOutput=False)
    w_d = nc.declare_dram_parameter("W", [COUT, CIN, 3, 3], F32, isOutput=False)
    b_d = nc.declare_dram_parameter("b", [COUT], F32, isOutput=False)
    id_d = nc.declare_dram_parameter("ident", [P, P], F32, isOutput=False)
    on_d = nc.declare_dram_parameter("ones_bf", [1, P], BF16, isOutput=False)
    tv_d = nc.declare_dram_parameter("tvt", [HP, H], BF16, isOutput=False)
    out_d = nc.declare_dram_parameter("out", [NL, COUT, H, W], F32, isOutput=True)

    with tile.TileContext(nc, num_cores=NCORES) as tc:
        with (
            tc.tile_pool(name="statics", bufs=1) as st,
            tc.tile_pool(name="xw", bufs=5) as xw,
            tc.tile_pool(name="axnp", bufs=4) as axnp,
            tc.tile_pool(name="smalls", bufs=2) as sm,
            tc.tile_pool(name="zp", bufs=3) as zp,
            tc.tile_pool(name="outp", bufs=4) as outp,
            tc.tile_pool(name="ps_conv", bufs=4, space="PSUM") as ps_conv,
            tc.tile_pool(name="ps_small", bufs=2, space="PSUM") as ps_small,
            tc.tile_pool(name="ps_bc", bufs=2, space="PSUM") as ps_bc,
            tc.tile_pool(name="dram", bufs=1, space="DRAM") as dr,
        ):
            # ---------------- static buffers (zeroed pads) ----------------
            # xq holds sign(xn) in fp8 for BOTH channel halves: [P, ko=2, FREEPAD]
            xq = st.tile([P, 2 * FREEPAD], FP8, name="xq", tag="xq")
            # zero only the positions the conv taps read but sign() never
            # writes: the dead margins, the pad rows, and the pad cols.
            for ko in range(2):
                kb = ko * FREEPAD
                nc.vector.memset(xq[:, kb:kb + MARGIN], 0.0)
                nc.vector.memset(xq[:, kb + MARGIN + NL * IMGP: kb + FREEPAD], 0.0)
                for img in range(NL):
                    ib = kb + MARGIN + img * IMGP
                    nc.vector.memset(xq[:, ib:ib + WP], 0.0)                # pad row 0
                    nc.vector.memset(xq[:, ib + (HP - 1) * WP: ib + IMGP], 0.0)  # pad row 57
                    colv = (xq[:, ib + WP: ib + (HP - 1) * WP]
                            .rearrange("p (h w) -> p h w", w=WP))
                    nc.vector.memset(colv[:, :, 0:1], 0.0)                  # pad col 0
                    nc.vector.memset(colv[:, :, WP - 1:WP], 0.0)            # pad col 57
            m_flat = []
            for img in range(NL):
                mf = st.tile([1, IMGP], BF16, name=f"mflat{img}", tag=f"mflat{img}")
                nc.vector.memset(mf[:, 0:WP], 0.0)
                nc.vector.memset(mf[:, (HP - 1) * WP:IMGP], 0.0)
                mfv = mf[:, WP:(HP - 1) * WP].rearrange("p (h w) -> p h w", w=WP)
                nc.vector.memset(mfv[:, :, 0:1], 0.0)
                nc.vector.memset(mfv[:, :, WP - 1:WP], 0.0)
                m_flat.append(mf)

            # ---------------- warmup collective ----------------
            # The first collective pays ncfw setup + absorbs cross-core start
            # skew; run a throwaway AllGather immediately so the real one
            # (after BN stats) finds warm ncfw state and synchronized peers.
            # Trigger with no data dependency (values are ignored) so it
            # fires in the first microsecond of the kernel.
            wu_in = dr.tile([1, 8], F32, name="wu_in", tag="wu_in")
            wu_out = dr.tile([NCORES, 1, 8], F32, name="wu_out", tag="wu_out",
                             addr_space="Shared")
            nc.gpsimd.collective_compute(
                "AllGather", ALU.bypass,
                replica_groups=[list(range(NCORES))],
                ins=[wu_in.opt()], outs=[wu_out.opt()],
            )

            # ---------------- host constants ----------------
            ident = st.tile([P, P], F32, name="ident_sb", tag="ident_sb")
            nc.sync.dma_start(ident[:], id_d.ap())
            onesb = st.tile([1, P], BF16, name="onesb_sb", tag="onesb_sb")
            nc.sync.dma_start(onesb[:], on_d.ap())
            tvt = st.tile([HP, H], BF16, name="tvt_sb", tag="tvt_sb")
            nc.sync.dma_start(tvt[:], tv_d.ap())

            gam, bet = [], []
            for kc in range(2):
                g = st.tile([P, 1], F32, name=f"gam{kc}", tag=f"gam{kc}")
                nc.sync.dma_start(g[:], g_d.ap()[kc * P:(kc + 1) * P][:, None])
                gam.append(g)
                be = st.tile([P, 1], F32, name=f"bet{kc}", tag=f"bet{kc}")
                nc.sync.dma_start(be[:], bb_d.ap()[kc * P:(kc + 1) * P][:, None])
                bet.append(be)
            bsb = []
            for oc in range(2):
                bt = st.tile([P, 1], F32, name=f"bsb{oc}", tag=f"bsb{oc}")
                nc.sync.dma_start(bt[:], b_d.ap()[oc * P:(oc + 1) * P][:, None])
                bsb.append(bt)

            # ---------------- weight prep ----------------
            w_nat = []
            for oc in range(2):
                wn = xw.tile([P, NPIX], F32, name="w_nat", tag="xw")
                nc.sync.dma_start(
                    wn[:, 0:CIN * KTAPS],
                    w_d.ap()[oc * P:(oc + 1) * P].rearrange("o c kh kw -> o (c kh kw)"),
                )
                w_nat.append(wn)
            alpha_sc, ab = [], []
            for oc in range(2):
                araw = st.tile([P, 1], F32, name=f"alph_raw{oc}", tag=f"alph_raw{oc}")
                nc.vector.tensor_reduce(
                    araw[:], w_nat[oc][:, 0:CIN * KTAPS], axis=AX.X, op=ALU.add,
                    apply_absolute_value=True,
                )
                asc = st.tile([P, 1], F32, name=f"alph{oc}", tag=f"alph{oc}")
                nc.vector.tensor_scalar_mul(asc[:], araw[:], 1.0 / (CIN * KTAPS))
                alpha_sc.append(asc)
                abt = st.tile([P, 1], F32, name=f"ab{oc}", tag=f"ab{oc}")
                nc.vector.tensor_mul(abt[:], asc[:], bsb[oc][:])
                ab.append(abt)

            # wq: sign(W) transposed into DoubleRow lhsT layout
            # [P(ki), tap, oc, ko, m] with ko = channel half (c = ko*128+ki)
            wq = st.tile([P, KTAPS * 2 * 2 * P], FP8, name="wq", tag="wq")
            wqv = wq.rearrange("p (t o k m) -> p t o k m", t=KTAPS, o=2, k=2)
            for oc in range(2):
                wv = w_nat[oc][:, 0:CIN * KTAPS].rearrange("o (c t) -> o c t", t=KTAPS)
                for kc in range(2):
                    for tap in range(KTAPS):
                        psT = ps_conv.tile([P, P], F32, name="psT", tag="ps_conv")
                        nc.tensor.transpose(psT[:], wv[:, kc * P:(kc + 1) * P, tap], ident[:])
                        nc.scalar.activation(wqv[:, tap, oc, kc, :], psT[:], AF.Sign)

            # ---------------- pass 1: BN statistics ----------------
            stats = []
            for kc in range(2):
                sb = st.tile([P, NL * NCH * 6], F32, name=f"stats{kc}", tag=f"stats{kc}")
                stats.append(sb)
            for img in range(NL):
                for kc in range(2):
                    xt = xw.tile([P, NPIX], F32, name="xt", tag="xw")
                    nc.sync.dma_start(
                        xt[:], x_d.ap()[img, kc * P:(kc + 1) * P].rearrange("c h w -> c (h w)")
                    )
                    for g in range(NCH):
                        col = (img * NCH + g) * 6
                        nc.vector.bn_stats(
                            stats[kc][:, col:col + 6],
                            xt[:, g * CF:(g + 1) * CF],
                        )
            pay = st.tile([P, 4], F32, name="pay", tag="pay")
            for kc in range(2):
                ag = st.tile([P, 2], F32, name=f"agg{kc}", tag=f"agg{kc}")
                nc.vector.bn_aggr(ag[:], stats[kc][:])
                nc.vector.tensor_copy(pay[:, 2 * kc:2 * kc + 1], ag[:, 0:1])
                msq = st.tile([P, 1], F32, name=f"msq{kc}", tag=f"msq{kc}")
                nc.vector.tensor_mul(msq[:], ag[:, 0:1], ag[:, 0:1])
                # E[x^2] per core = var + mean^2
                nc.vector.tensor_add(pay[:, 2 * kc + 1:2 * kc + 2], ag[:, 1:2], msq[:])

            cc_in = dr.tile([P, 4], F32, name="cc_in", tag="cc_in")
            cc_out = dr.tile([NCORES, P, 4], F32, name="cc_out", tag="cc_out",
                             addr_space="Shared")
            nc.sync.dma_start(cc_in[:], pay[:])
            nc.gpsimd.collective_compute(
                "AllGather", ALU.bypass,
                replica_groups=[list(range(NCORES))],
                ins=[cc_in.opt()], outs=[cc_out.opt()],
            )
            ag_sb = st.tile([P, NCORES * 4], F32, name="ag_sb", tag="ag_sb")
            nc.sync.dma_start(
                ag_sb[:].rearrange("p (r c) -> p r c", c=4),
                cc_out.rearrange("r p c -> p r c"),
            )
            # local sum over the 8 ranks -> same [P, 4] view the AR produced
            arsb = st.tile([P, 4], F32, name="arsb", tag="arsb")
            nc.vector.tensor_reduce(
                arsb[:],
                ag_sb[:].rearrange("p (r c) -> p c r", c=4),
                axis=AX.X, op=ALU.add,
            )

            # ---------------- global BN scalars ----------------
            epsc = st.tile([P, 1], F32, name="epsc", tag="epsc")
            nc.vector.memset(epsc[:], EPS)
            svec_bf, tprime = [], []
            for kc in range(2):
                mu = st.tile([P, 1], F32, name=f"mu{kc}", tag=f"mu{kc}")
                nc.vector.tensor_scalar_mul(mu[:], arsb[:, 2 * kc:2 * kc + 1], 1.0 / NCORES)
                ex2 = st.tile([P, 1], F32, name=f"ex2{kc}", tag=f"ex2{kc}")
                nc.vector.tensor_scalar_mul(ex2[:], arsb[:, 2 * kc + 1:2 * kc + 2], 1.0 / NCORES)
                msq2 = st.tile([P, 1], F32, name=f"musq{kc}", tag=f"musq{kc}")
                nc.vector.tensor_mul(msq2[:], mu[:], mu[:])
                var = st.tile([P, 1], F32, name=f"var{kc}", tag=f"var{kc}")
                nc.vector.tensor_sub(var[:], ex2[:], msq2[:])
                sig = st.tile([P, 1], F32, name=f"sig{kc}", tag=f"sig{kc}")
                nc.scalar.activation(sig[:], var[:], AF.Sqrt, bias=epsc[:])
                rsig = st.tile([P, 1], F32, name=f"rsig{kc}", tag=f"rsig{kc}")
                nc.vector.reciprocal(rsig[:], sig[:])
                s = st.tile([P, 1], F32, name=f"s{kc}", tag=f"s{kc}")
                nc.vector.tensor_mul(s[:], gam[kc][:], rsig[:])
                rg = st.tile([P, 1], F32, name=f"rg{kc}", tag=f"rg{kc}")
                nc.vector.reciprocal(rg[:], gam[kc][:])
                tb = st.tile([P, 1], F32, name=f"tb{kc}", tag=f"tb{kc}")
                nc.vector.tensor_mul(tb[:], bet[kc][:], sig[:])
                tb2 = st.tile([P, 1], F32, name=f"tb2{kc}", tag=f"tb2{kc}")
                nc.vector.tensor_mul(tb2[:], tb[:], rg[:])
                tp = st.tile([P, 1], F32, name=f"tp{kc}", tag=f"tp{kc}")
                nc.vector.tensor_sub(tp[:], tb2[:], mu[:])
                tprime.append(tp)
                smf = st.tile([P, 1], F32, name=f"smf{kc}", tag=f"smf{kc}")
                nc.vector.tensor_scalar_mul(smf[:], s[:], 1.0 / CIN)
                smb = st.tile([P, 1], BF16, name=f"smb{kc}", tag=f"smb{kc}")
                nc.vector.tensor_copy(smb[:], smf[:])
                svec_bf.append(smb)

            # ---------------- pass 2: binarize + conv, per image ----------------
            for img in range(NL):
                axn = []
                for kc in range(2):
                    xt2 = xw.tile([P, NPIX], F32, name="xt2", tag="xw")
                    nc.sync.dma_start(
                        xt2[:], x_d.ap()[img, kc * P:(kc + 1) * P].rearrange("c h w -> c (h w)")
                    )
                    ax = axnp.tile([P, NPIX], BF16, name="ax", tag="ax")
                    nc.scalar.activation(ax[:], xt2[:], AF.Abs, bias=tprime[kc][:])
                    xqv = (xq[:, kc * FREEPAD + MARGIN + img * IMGP:
                              kc * FREEPAD + MARGIN + (img + 1) * IMGP]
                           .rearrange("p (h w) -> p h w", w=WP))
                    nc.scalar.activation(
                        xqv[:, 1:1 + H, 1:1 + W],
                        xt2.rearrange("p (h w) -> p h w", w=W),
                        AF.Sign, bias=tprime[kc][:],
                    )
                    axn.append(ax)

                # channel-mean magnitudes m (weighted column sums via PE)
                for ch in range(NCH):
                    mp = ps_small.tile([1, CF], F32, name="mps", tag="ps_small")
                    nc.tensor.matmul(mp[:], svec_bf[0][:], axn[0][:, ch * CF:(ch + 1) * CF],
                                     start=True, stop=False)
                    nc.tensor.matmul(mp[:], svec_bf[1][:], axn[1][:, ch * CF:(ch + 1) * CF],
                                     start=False, stop=True)
                    mfv = m_flat[img].rearrange("p (h w) -> p h w", w=WP)
                    nc.scalar.activation(
                        mfv[:, 1 + ch * CH_ROWS: 1 + (ch + 1) * CH_ROWS, 1:1 + W],
                        mp.rearrange("p (h w) -> p h w", w=W),
                        AF.Copy,
                    )

                # beta_map = box3x3(m): horizontal on DVE, vertical via banded matmul
                mhw = sm.tile([HP, WP], BF16, name="mhw", tag="mhw")
                nc.sync.dma_start(mhw[:], m_flat[img][:])
                hs = sm.tile([HP, WP], BF16, name="hs", tag="hs")
                nc.vector.tensor_add(hs[:, 1:1 + W], mhw[:, 0:W], mhw[:, 2:2 + W])
                nc.vector.tensor_add(hs[:, 1:1 + W], hs[:, 1:1 + W], mhw[:, 1:1 + W])
                bps = ps_small.tile([H, W], F32, name="bps", tag="ps_small")
                nc.tensor.matmul(bps[:], tvt[:], hs[:, 1:1 + W], start=True, stop=True)
                bhw = sm.tile([H, W], BF16, name="bhw", tag="bhw")
                nc.vector.tensor_copy(bhw[:], bps[:])
                bflat = sm.tile([1, NPIX], BF16, name="bflat", tag="bflat")
                nc.sync.dma_start(bflat[:], bhw[:])

                # binary conv + epilogue
                for ch in range(NCH):
                    bcp = ps_bc.tile([P, CF], F32, name="bcp", tag="ps_bc")
                    nc.tensor.matmul(bcp[:], onesb[:], bflat[:, ch * CF:(ch + 1) * CF],
                                     start=True, stop=True)
                    base = MARGIN + img * IMGP + (1 + ch * CH_ROWS) * WP
                    xq2 = xq[:].rearrange("p (k f) -> p k f", k=2)
                    for oc in range(2):
                        cv = ps_conv.tile([P, CFP], F32, name="cv", tag="ps_conv")
                        for tap in range(KTAPS):
                            dh, dw = tap // 3, tap % 3
                            off = (dh - 1) * WP + (dw - 1)
                            nc.tensor.matmul(
                                cv[:],
                                wqv[:, tap, oc],
                                xq2[:, :, base + off: base + off + CFP],
                                start=(tap == 0), stop=(tap == KTAPS - 1),
                                perf_mode=mybir.MatmulPerfMode.DoubleRow,
                            )
                        z = zp.tile([P, CF], F32, name="z", tag="z")
                        cvv = cv.rearrange("p (h w) -> p h w", w=WP)
                        nc.scalar.activation(
                            z.rearrange("p (h w) -> p h w", w=W),
                            cvv[:, :, 1:1 + W],
                            AF.Relu, bias=ab[oc][:], scale=alpha_sc[oc][:],
                        )
                        ot = outp.tile([P, CF], F32, name="ot", tag="ot")
                        nc.vector.tensor_mul(ot[:], z[:], bcp[:])
                        nc.sync.dma_start(
                            out_d.ap()[img, oc * P:(oc + 1) * P,
                                       ch * CH_ROWS:(ch + 1) * CH_ROWS, :],
                            ot.rearrange("p (h w) -> p h w", w=W),
                        )

    nc.compile()
    return nc


_NC_CACHE: dict = {}


def _get_nc(n_local: int):
    if n_local not in _NC_CACHE:
        _NC_CACHE[n_local] = _build(n_local)
    return _NC_CACHE[n_local]


def _host_consts():
    ident = np.eye(P, dtype=np.float32)
    ones_bf = np.ones((1, P), dtype=NPBF16)
    tvt = np.zeros((HP, H), dtype=np.float32)
    for h in range(H):
        tvt[h:h + 3, h] = 1.0 / 9.0
    return ident, ones_bf, tvt.astype(NPBF16)


def _run(inputs: dict, trace: bool = False):
    x = np.ascontiguousarray(np.asarray(inputs["x"], dtype=np.float32))
    gamma = np.ascontiguousarray(np.asarray(inputs["gamma"], dtype=np.float32))
    beta_bn = np.ascontiguousarray(np.asarray(inputs["beta_bn"], dtype=np.float32))
    Wt = np.ascontiguousarray(np.asarray(inputs["W"], dtype=np.float32))
    b = np.ascontiguousarray(np.asarray(inputs["b"], dtype=np.float32))

    n = x.shape[0]
    assert n % NCORES == 0, f"batch {n} not divisible by {NCORES}"
    nl = n // NCORES
    nc = _get_nc(nl)
    ident, ones_bf, tvt = _host_consts()

    in_maps = []
    for i in range(NCORES):
        in_maps.append({
            "x": np.ascontiguousarray(x[i * nl:(i + 1) * nl]),
            "gamma": gamma, "beta_bn": beta_bn, "W": Wt, "b": b,
            "ident": ident, "ones_bf": ones_bf, "tvt": tvt,
        })
    res = run_bass_kernel_spmd(nc, in_maps, core_ids=list(range(NCORES)),
                               trace=trace)
    out = np.concatenate([res.results[i]["out"] for i in range(NCORES)], axis=0)
    return out, res


def kernel(**inputs) -> np.ndarray:
    out, _ = _run(inputs, trace=False)
    return out


def kernel_timed(**inputs):
    out, res = _run(inputs, trace=True)
    return out, res


# revision 18
# speedup vs baseline: 1.6249x; 1.0055x over previous
"""BinConv2d (XNOR-Net style) Trainium2 kernel, 8-core data-parallel.

Layer math (BatchNorm train-mode -> BinActiv -> binary 3x3 conv -> scale by
box-filtered channel-mean magnitudes and per-filter alpha -> relu):

  mu, var: batch stats of x over (N, H, W) per channel      (needs all-reduce)
  xn  = (x - mu) * rsqrt(var + eps) * gamma + beta
  m   = mean_c |xn|;  xb = sign(xn);  Wb = sign(W);  alpha = mean |W| per filter
  y   = conv(xb, Wb, pad=1) + b
  out = relu(y * box3x3(m) * alpha)

Key implementation facts:
  - xb, Wb are exactly representable in bf16; matmuls accumulate fp32 in PSUM,
    so the binary conv result is EXACT integers.
  - sign(xn) = sign(x + t') with t' = beta*sigma/gamma - mu  (gamma > 0), and
    |xn| = s*|x + t'| with s = gamma*rsig folded into the m-matmul weights.
  - 3x3 conv = 9 shifted matmuls over a zero-padded [C, 58*58] flat layout;
    every tap is a pure 1D offset, pad columns absorb edge effects.
"""

import os
import sys

import numpy as np

for _p in ("/opt/trn_rl_repo", "/root/.axon_site/_ro/trn_rl_repo"):
    if os.path.isdir(_p) and _p not in sys.path:
        sys.path.insert(0, _p)

import concourse.bass as bass  # noqa: E402
import concourse.bacc as bacc  # noqa: E402
import concourse.mybir as mybir  # noqa: E402
import concourse.tile as tile  # noqa: E402
from concourse.bass_utils import run_bass_kernel_spmd  # noqa: E402

F32 = mybir.dt.float32
BF16 = mybir.dt.bfloat16
FP8 = mybir.dt.float8e4
NPBF16 = mybir.dt.np(BF16)
AF = mybir.ActivationFunctionType
ALU = mybir.AluOpType
AX = mybir.AxisListType

EPS = 1e-4
NCORES = 8
P = 128
CIN = 256
COUT = 256
H = 56
W = 56
HP = H + 2          # 58 padded rows
WP = W + 2          # 58 padded cols
IMGP = HP * WP      # 3364 padded pixels / image
NPIX = H * W        # 3136 true pixels / image
MARGIN = 64         # dead zero margin absorbing out-of-image tap reads
CH_ROWS = 8         # output rows per PSUM chunk
NCH = H // CH_ROWS  # 7 chunks
CF = CH_ROWS * W    # 448 compact free elems / chunk
CFP = CH_ROWS * WP  # 464 padded free elems / chunk
KTAPS = 9


def _build(n_local: int):
    """Build the SPMD program for n_local images per core."""
    NL = n_local
    FREEPAD = 2 * MARGIN + NL * IMGP

    nc = bacc.Bacc("TRN2", debug=False, target_bir_lowering=False,
                   num_devices=NCORES)
    x_d = nc.declare_dram_parameter("x", [NL, CIN, H, W], F32, isOutput=False)
    g_d = nc.declare_dram_parameter("gamma", [CIN], F32, isOutput=False)
    bb_d = nc.declare_dram_parameter("beta_bn", [CIN], F32, isOutput=False)
    w_d = nc.declare_dram_parameter("W", [COUT, CIN, 3, 3], F32, isOutput=False)
    b_d = nc.declare_dram_parameter("b", [COUT], F32, isOutput=False)
    id_d = nc.declare_dram_parameter("ident", [P, P], F32, isOutput=False)
    on_d = nc.declare_dram_parameter("ones_bf", [1, P], BF16, isOutput=False)
    tv_d = nc.declare_dram_parameter("tvt", [HP, H], BF16, isOutput=False)
    out_d = nc.declare_dram_parameter("out", [NL, COUT, H, W], F32, isOutput=True)

    with tile.TileContext(nc, num_cores=NCORES) as tc:
        with (
            tc.tile_pool(name="statics", bufs=1) as st,
            tc.tile_pool(name="xw", bufs=5) as xw,
            tc.tile_pool(name="axnp", bufs=4) as axnp,
            tc.tile_pool(name="smalls", bufs=2) as sm,
            tc.tile_pool(name="zp", bufs=3) as zp,
            tc.tile_pool(name="outp", bufs=4) as outp,
            tc.tile_pool(name="ps_conv", bufs=4, space="PSUM") as ps_conv,
            tc.tile_pool(name="ps_small", bufs=2, space="PSUM") as ps_small,
            tc.tile_pool(name="ps_bc", bufs=2, space="PSUM") as ps_bc,
            tc.tile_pool(name="dram", bufs=1, space="DRAM") as dr,
        ):
            # ---------------- static buffers (zeroed pads) ----------------
            # xq holds sign(xn) in fp8 for BOTH channel halves: [P, ko=2, FREEPAD]
            xq = st.tile([P, 2 * FREEPAD], FP8, name="xq", tag="xq")
            # zero only the positions the conv taps read but sign() never
            # writes: the dead margins, the pad rows, and the pad cols.
            for ko in range(2):
                kb = ko * FREEPAD
                nc.vector.memset(xq[:, kb:kb + MARGIN], 0.0)
                nc.vector.memset(xq[:, kb + MARGIN + NL * IMGP: kb + FREEPAD], 0.0)
                for img in range(NL):
                    ib = kb + MARGIN + img * IMGP
                    nc.vector.memset(xq[:, ib:ib + WP], 0.0)                # pad row 0
                    nc.vector.memset(xq[:, ib + (HP - 1) * WP: ib + IMGP], 0.0)  # pad row 57
                    colv = (xq[:, ib + WP: ib + (HP - 1) * WP]
                            .rearrange("p (h w) -> p h w", w=WP))
                    nc.vector.memset(colv[:, :, 0:1], 0.0)                  # pad col 0
                    nc.vector.memset(colv[:, :, WP - 1:WP], 0.0)            # pad col 57
            m_flat = []
            for img in range(NL):
                mf = st.tile([1, IMGP], BF16, name=f"mflat{img}", tag=f"mflat{img}")
                nc.vector.memset(mf[:, 0:WP], 0.0)
                nc.vector.memset(mf[:, (HP - 1) * WP:IMGP], 0.0)
                mfv = mf[:, WP:(HP - 1) * WP].rearrange("p (h w) -> p h w", w=WP)
                nc.vector.memset(mfv[:, :, 0:1], 0.0)
                nc.vector.memset(mfv[:, :, WP - 1:WP], 0.0)
                m_flat.append(mf)

            # ---------------- warmup collective ----------------
            # The first collective pays ncfw setup + absorbs cross-core start
            # skew; run a throwaway AllGather immediately so the real one
            # (after BN stats) finds warm ncfw state and synchronized peers.
            # Trigger with no data dependency (values are ignored) so it
            # fires in the first microsecond of the kernel.
            wu_in = dr.tile([1, 8], F32, name="wu_in", tag="wu_in")
            wu_out = dr.tile([NCORES, 1, 8], F32, name="wu_out", tag="wu_out",
                             addr_space="Shared")
            nc.gpsimd.collective_compute(
                "AllGather", ALU.bypass,
                replica_groups=[list(range(NCORES))],
                ins=[wu_in.opt()], outs=[wu_out.opt()],
            )

            # ---------------- host constants ----------------
            ident = st.tile([P, P], F32, name="ident_sb", tag="ident_sb")
            nc.sync.dma_start(ident[:], id_d.ap())
            onesb = st.tile([1, P], BF16, name="onesb_sb", tag="onesb_sb")
            nc.sync.dma_start(onesb[:], on_d.ap())
            tvt = st.tile([HP, H], BF16, name="tvt_sb", tag="tvt_sb")
            nc.sync.dma_start(tvt[:], tv_d.ap())

            gam, bet = [], []
            for kc in range(2):
                g = st.tile([P, 1], F32, name=f"gam{kc}", tag=f"gam{kc}")
                nc.sync.dma_start(g[:], g_d.ap()[kc * P:(kc + 1) * P][:, None])
                gam.append(g)
                be = st.tile([P, 1], F32, name=f"bet{kc}", tag=f"bet{kc}")
                nc.sync.dma_start(be[:], bb_d.ap()[kc * P:(kc + 1) * P][:, None])
                bet.append(be)
            bsb = []
            for oc in range(2):
                bt = st.tile([P, 1], F32, name=f"bsb{oc}", tag=f"bsb{oc}")
                nc.sync.dma_start(bt[:], b_d.ap()[oc * P:(oc + 1) * P][:, None])
                bsb.append(bt)

            # ---------------- weight prep ----------------
            w_nat = []
            for oc in range(2):
                wn = xw.tile([P, NPIX], F32, name="w_nat", tag="xw")
                nc.sync.dma_start(
                    wn[:, 0:CIN * KTAPS],
                    w_d.ap()[oc * P:(oc + 1) * P].rearrange("o c kh kw -> o (c kh kw)"),
                )
                w_nat.append(wn)
            alpha_sc, ab = [], []
            for oc in range(2):
                araw = st.tile([P, 1], F32, name=f"alph_raw{oc}", tag=f"alph_raw{oc}")
                nc.vector.tensor_reduce(
                    araw[:], w_nat[oc][:, 0:CIN * KTAPS], axis=AX.X, op=ALU.add,
                    apply_absolute_value=True,
                )
                asc = st.tile([P, 1], F32, name=f"alph{oc}", tag=f"alph{oc}")
                nc.vector.tensor_scalar_mul(asc[:], araw[:], 1.0 / (CIN * KTAPS))
                alpha_sc.append(asc)
                abt = st.tile([P, 1], F32, name=f"ab{oc}", tag=f"ab{oc}")
                nc.vector.tensor_mul(abt[:], asc[:], bsb[oc][:])
                ab.append(abt)

            # wq: sign(W) transposed into DoubleRow lhsT layout
            # [P(ki), tap, oc, ko, m] with ko = channel half (c = ko*128+ki)
            wq = st.tile([P, KTAPS * 2 * 2 * P], FP8, name="wq", tag="wq")
            wqv = wq.rearrange("p (t o k m) -> p t o k m", t=KTAPS, o=2, k=2)
            for oc in range(2):
                wv = w_nat[oc][:, 0:CIN * KTAPS].rearrange("o (c t) -> o c t", t=KTAPS)
                for kc in range(2):
                    for tap in range(KTAPS):
                        psT = ps_conv.tile([P, P], F32, name="psT", tag="ps_conv")
                        nc.tensor.transpose(psT[:], wv[:, kc * P:(kc + 1) * P, tap], ident[:])
                        nc.scalar.activation(wqv[:, tap, oc, kc, :], psT[:], AF.Sign)

            # ---------------- pass 1: BN statistics ----------------
            stats = []
            for kc in range(2):
                sb = st.tile([P, NL * NCH * 6], F32, name=f"stats{kc}", tag=f"stats{kc}")
                stats.append(sb)
            for img in range(NL):
                for kc in range(2):
                    xt = xw.tile([P, NPIX], F32, name="xt", tag="xw")
                    deng = nc.sync if kc == 0 else nc.scalar
                    deng.dma_start(
                        xt[:], x_d.ap()[img, kc * P:(kc + 1) * P].rearrange("c h w -> c (h w)")
                    )
                    for g in range(NCH):
                        col = (img * NCH + g) * 6
                        nc.vector.bn_stats(
                            stats[kc][:, col:col + 6],
                            xt[:, g * CF:(g + 1) * CF],
                        )
            pay = st.tile([P, 4], F32, name="pay", tag="pay")
            for kc in range(2):
                ag = st.tile([P, 2], F32, name=f"agg{kc}", tag=f"agg{kc}")
                nc.vector.bn_aggr(ag[:], stats[kc][:])
                nc.vector.tensor_copy(pay[:, 2 * kc:2 * kc + 1], ag[:, 0:1])
                msq = st.tile([P, 1], F32, name=f"msq{kc}", tag=f"msq{kc}")
                nc.vector.tensor_mul(msq[:], ag[:, 0:1], ag[:, 0:1])
                # E[x^2] per core = var + mean^2
                nc.vector.tensor_add(pay[:, 2 * kc + 1:2 * kc + 2], ag[:, 1:2], msq[:])

            cc_in = dr.tile([P, 4], F32, name="cc_in", tag="cc_in")
            cc_out = dr.tile([NCORES, P, 4], F32, name="cc_out", tag="cc_out",
                             addr_space="Shared")
            nc.sync.dma_start(cc_in[:], pay[:])
            nc.gpsimd.collective_compute(
                "AllGather", ALU.bypass,
                replica_groups=[list(range(NCORES))],
                ins=[cc_in.opt()], outs=[cc_out.opt()],
            )
            ag_sb = st.tile([P, NCORES * 4], F32, name="ag_sb", tag="ag_sb")
            nc.sync.dma_start(
                ag_sb[:].rearrange("p (r c) -> p r c", c=4),
                cc_out.rearrange("r p c -> p r c"),
            )
            # local sum over the 8 ranks -> same [P, 4] view the AR produced
            arsb = st.tile([P, 4], F32, name="arsb", tag="arsb")
            nc.vector.tensor_reduce(
                arsb[:],
                ag_sb[:].rearrange("p (r c) -> p c r", c=4),
                axis=AX.X, op=ALU.add,
            )

            # ---------------- global BN scalars ----------------
            epsc = st.tile([P, 1], F32, name="epsc", tag="epsc")
            nc.vector.memset(epsc[:], EPS)
            svec_bf, tprime = [], []
            for kc in range(2):
                mu = st.tile([P, 1], F32, name=f"mu{kc}", tag=f"mu{kc}")
                nc.vector.tensor_scalar_mul(mu[:], arsb[:, 2 * kc:2 * kc + 1], 1.0 / NCORES)
                ex2 = st.tile([P, 1], F32, name=f"ex2{kc}", tag=f"ex2{kc}")
                nc.vector.tensor_scalar_mul(ex2[:], arsb[:, 2 * kc + 1:2 * kc + 2], 1.0 / NCORES)
                msq2 = st.tile([P, 1], F32, name=f"musq{kc}", tag=f"musq{kc}")
                nc.vector.tensor_mul(msq2[:], mu[:], mu[:])
                var = st.tile([P, 1], F32, name=f"var{kc}", tag=f"var{kc}")
                nc.vector.tensor_sub(var[:], ex2[:], msq2[:])
                sig = st.tile([P, 1], F32, name=f"sig{kc}", tag=f"sig{kc}")
                nc.scalar.activation(sig[:], var[:], AF.Sqrt, bias=epsc[:])
                rsig = st.tile([P, 1], F32, name=f"rsig{kc}", tag=f"rsig{kc}")
                nc.vector.reciprocal(rsig[:], sig[:])
                s = st.tile([P, 1], F32, name=f"s{kc}", tag=f"s{kc}")
                nc.vector.tensor_mul(s[:], gam[kc][:], rsig[:])
                rg = st.tile([P, 1], F32, name=f"rg{kc}", tag=f"rg{kc}")
                nc.vector.reciprocal(rg[:], gam[kc][:])
                tb = st.tile([P, 1], F32, name=f"tb{kc}", tag=f"tb{kc}")
                nc.vector.tensor_mul(tb[:], bet[kc][:], sig[:])
                tb2 = st.tile([P, 1], F32, name=f"tb2{kc}", tag=f"tb2{kc}")
                nc.vector.tensor_mul(tb2[:], tb[:], rg[:])
                tp = st.tile([P, 1], F32, name=f"tp{kc}", tag=f"tp{kc}")
                nc.vector.tensor_sub(tp[:], tb2[:], mu[:])
                tprime.append(tp)
                smf = st.tile([P, 1], F32, name=f"smf{kc}", tag=f"smf{kc}")
                nc.vector.tensor_scalar_mul(smf[:], s[:], 1.0 / CIN)
                smb = st.tile([P, 1], BF16, name=f"smb{kc}", tag=f"smb{kc}")
                nc.vector.tensor_copy(smb[:], smf[:])
                svec_bf.append(smb)

            # ---------------- pass 2: binarize + conv, software-pipelined ----
            # Emission order sets per-engine priority: emit image i+1's
            # binarize/m-path BEFORE image i's conv so ACT/DVE prefetch work
            # for the next image instead of queueing behind this image's
            # epilogue.
            bflats = {}

            def binarize(img):
                axn = []
                for kc in range(2):
                    xt2 = xw.tile([P, NPIX], F32, name="xt2", tag="xw")
                    deng = nc.sync if kc == 0 else nc.scalar
                    deng.dma_start(
                        xt2[:], x_d.ap()[img, kc * P:(kc + 1) * P].rearrange("c h w -> c (h w)")
                    )
                    ax = axnp.tile([P, NPIX], BF16, name="ax", tag="ax")
                    nc.scalar.activation(ax[:], xt2[:], AF.Abs, bias=tprime[kc][:])
                    xqv = (xq[:, kc * FREEPAD + MARGIN + img * IMGP:
                              kc * FREEPAD + MARGIN + (img + 1) * IMGP]
                           .rearrange("p (h w) -> p h w", w=WP))
                    nc.scalar.activation(
                        xqv[:, 1:1 + H, 1:1 + W],
                        xt2.rearrange("p (h w) -> p h w", w=W),
                        AF.Sign, bias=tprime[kc][:],
                    )
                    axn.append(ax)

                # channel-mean magnitudes m (weighted column sums via PE)
                for ch in range(NCH):
                    mp = ps_small.tile([1, CF], F32, name="mps", tag="ps_small")
                    nc.tensor.matmul(mp[:], svec_bf[0][:], axn[0][:, ch * CF:(ch + 1) * CF],
                                     start=True, stop=False)
                    nc.tensor.matmul(mp[:], svec_bf[1][:], axn[1][:, ch * CF:(ch + 1) * CF],
                                     start=False, stop=True)
                    mfv = m_flat[img].rearrange("p (h w) -> p h w", w=WP)
                    nc.scalar.activation(
                        mfv[:, 1 + ch * CH_ROWS: 1 + (ch + 1) * CH_ROWS, 1:1 + W],
                        mp.rearrange("p (h w) -> p h w", w=W),
                        AF.Copy,
                    )

                # beta_map = box3x3(m): horizontal on DVE, vertical via banded matmul
                mhw = sm.tile([HP, WP], BF16, name="mhw", tag="mhw")
                nc.sync.dma_start(mhw[:], m_flat[img][:])
                hs = sm.tile([HP, WP], BF16, name="hs", tag="hs")
                nc.vector.tensor_add(hs[:, 1:1 + W], mhw[:, 0:W], mhw[:, 2:2 + W])
                nc.vector.tensor_add(hs[:, 1:1 + W], hs[:, 1:1 + W], mhw[:, 1:1 + W])
                bps = ps_small.tile([H, W], F32, name="bps", tag="ps_small")
                nc.tensor.matmul(bps[:], tvt[:], hs[:, 1:1 + W], start=True, stop=True)
                bhw = sm.tile([H, W], BF16, name="bhw", tag="bhw")
                nc.vector.tensor_copy(bhw[:], bps[:])
                bflat = sm.tile([1, NPIX], BF16, name="bflat", tag="bflat", bufs=3)
                nc.sync.dma_start(bflat[:], bhw[:])
                bflats[img] = bflat

            def conv_img(img):
                bflat = bflats.pop(img)
                for ch in range(NCH):
                    bcp = ps_bc.tile([P, CF], F32, name="bcp", tag="ps_bc")
                    nc.tensor.matmul(bcp[:], onesb[:], bflat[:, ch * CF:(ch + 1) * CF],
                                     start=True, stop=True)
                    base = MARGIN + img * IMGP + (1 + ch * CH_ROWS) * WP
                    xq2 = xq[:].rearrange("p (k f) -> p k f", k=2)
                    for oc in range(2):
                        cv = ps_conv.tile([P, CFP], F32, name="cv", tag="ps_conv")
                        for tap in range(KTAPS):
                            dh, dw = tap // 3, tap % 3
                            off = (dh - 1) * WP + (dw - 1)
                            nc.tensor.matmul(
                                cv[:],
                                wqv[:, tap, oc],
                                xq2[:, :, base + off: base + off + CFP],
                                start=(tap == 0), stop=(tap == KTAPS - 1),
                                perf_mode=mybir.MatmulPerfMode.DoubleRow,
                            )
                        z = zp.tile([P, CF], F32, name="z", tag="z")
                        cvv = cv.rearrange("p (h w) -> p h w", w=WP)
                        nc.scalar.activation(
                            z.rearrange("p (h w) -> p h w", w=W),
                            cvv[:, :, 1:1 + W],
                            AF.Relu, bias=ab[oc][:], scale=alpha_sc[oc][:],
                        )
                        ot = outp.tile([P, CF], F32, name="ot", tag="ot")
                        nc.vector.tensor_mul(ot[:], z[:], bcp[:])
                        nc.sync.dma_start(
                            out_d.ap()[img, oc * P:(oc + 1) * P,
                                       ch * CH_ROWS:(ch + 1) * CH_ROWS, :],
                            ot.rearrange("p (h w) -> p h w", w=W),
                        )

            binarize(0)
            for img in range(1, NL):
                binarize(img)
                conv_img(img - 1)
            conv_img(NL - 1)

    nc.compile()
    return nc


_NC_CACHE: dict = {}


def _get_nc(n_local: int):
    if n_local not in _NC_CACHE:
        _NC_CACHE[n_local] = _build(n_local)
    return _NC_CACHE[n_local]


def _host_consts():
    ident = np.eye(P, dtype=np.float32)
    ones_bf = np.ones((1, P), dtype=NPBF16)
    tvt = np.zeros((HP, H), dtype=np.float32)
    for h in range(H):
        tvt[h:h + 3, h] = 1.0 / 9.0
    return ident, ones_bf, tvt.astype(NPBF16)


def _run(inputs: dict, trace: bool = False):
    x = np.ascontiguousarray(np.asarray(inputs["x"], dtype=np.float32))
    gamma = np.ascontiguousarray(np.asarray(inputs["gamma"], dtype=np.float32))
    beta_bn = np.ascontiguousarray(np.asarray(inputs["beta_bn"], dtype=np.float32))
    Wt = np.ascontiguousarray(np.asarray(inputs["W"], dtype=np.float32))
    b = np.ascontiguousarray(np.asarray(inputs["b"], dtype=np.float32))

    n = x.shape[0]
    assert n % NCORES == 0, f"batch {n} not divisible by {NCORES}"
    nl = n // NCORES
    nc = _get_nc(nl)
    ident, ones_bf, tvt = _host_consts()

    in_maps = []
    for i in range(NCORES):
        in_maps.append({
            "x": np.ascontiguousarray(x[i * nl:(i + 1) * nl]),
            "gamma": gamma, "beta_bn": beta_bn, "W": Wt, "b": b,
            "ident": ident, "ones_bf": ones_bf, "tvt": tvt,
        })
    res = run_bass_kernel_spmd(nc, in_maps, core_ids=list(range(NCORES)),
                               trace=trace)
    out = np.concatenate([res.results[i]["out"] for i in range(NCORES)], axis=0)
    return out, res


def kernel(**inputs) -> np.ndarray:
    out, _ = _run(inputs, trace=False)
    return out


def kernel_timed(**inputs):
    out, res = _run(inputs, trace=True)
    return out, res
